# revision 57
# baseline (speedup 1.0000x reference)
"""Two-layer GCN (PyG GCNConv x2 with relu between) on 8 Trainium2 NeuronCores.

Math (per layer, A' = D^-1/2 (A + I) D^-1/2):
    h  = relu(A' (z @ W1) + b1)
    out = A' (h @ W2) + b2  ==  (A' h) @ W2 + b2      (aggregation commutes with the
                                                       feature-space linear map)
Both layers therefore aggregate 128-wide features only.

Distribution: nodes (and dst-partitioned edges) sharded across 8 cores;
weights replicated; per-layer AllGather of the (dinv-scaled) feature table in
bf16; per-core gather of source rows via bulk SWDGE dma_gather; segment-sum
realized as one-hot matmuls accumulating in PSUM.

The Bass program is specialized to the actual graph: per-(window, class)
chunk counts are compile-time constants derived from edge_index.
"""

import itertools

import numpy as np
import ml_dtypes

P = 128
NCORES = 8
NCLASS = 4          # src-range classes so relative gather indices fit int16
G = 4               # dst windows per gather group

BF16 = ml_dtypes.bfloat16

_PROGRAM_CACHE = {}


# ----------------------------------------------------------------- host prep


def _plan(edge_index, N):
    """Sort/partition edges; all compile-time metadata + per-core slot arrays.

    src "classes" are window-ranges: class r covers windows [RS[r], RE[r]) of
    every core.  The per-class table chunk (one AllGather each) is laid out
    [core, window-in-range, 128] so a chunk's AllGather can fire as soon as
    every core has produced that window range.
    """
    WPC = -(-N // (NCORES * P))            # windows per core
    SHARD = WPC * P
    NPAD = NCORES * SHARD

    # make the LAST range smallest: its AllGather chunk is the one whose
    # latency is exposed between phases
    last = max(WPC // NCLASS - 6, 1)
    rest = WPC - last
    b3, r3 = divmod(rest, NCLASS - 1)
    RW = np.array([b3 + (1 if r < r3 else 0) for r in range(NCLASS - 1)] + [last])
    RS = np.concatenate([[0], np.cumsum(RW)])[:NCLASS]
    RE = RS + RW
    TSIZE = (NCORES * RW * P).tolist()
    assert max(NCORES * RW * P) <= 32767
    range_of = np.repeat(np.arange(NCLASS), RW)          # [WPC] -> class

    src = np.asarray(edge_index[0], dtype=np.int64)
    dst = np.asarray(edge_index[1], dtype=np.int64)
    deg = np.bincount(dst, minlength=N).astype(np.float64) + 1.0

    # self-loops are added on-chip via an identity matmul over the local
    # window tile; only real edges go through the gather
    s2 = src
    d2 = dst

    def relidx(s):
        k = s // SHARD
        wloc = (s % SHARD) >> 7
        r = range_of[wloc]
        return (k * RW[r] + (wloc - RS[r])) * P + (s & (P - 1)), r

    win = d2 >> 7
    rel2, cls = relidx(s2)
    key = win * NCLASS + cls
    # secondary sort by table row: each SDMA engine then walks ascending
    # addresses within a gather call (HBM locality)
    order = np.lexsort((rel2, key))
    d2s = d2[order]

    NW = NPAD // P
    cellcnt = np.bincount(key, minlength=NW * NCLASS)
    cellstart = np.concatenate([[0], np.cumsum(cellcnt)]).astype(np.int64)
    counts_core = cellcnt.reshape(NCORES, WPC, NCLASS)
    chunks_wc = -(-counts_core.max(axis=0) // P)      # [WPC, NCLASS]

    groups = [list(range(g, min(g + G, WPC))) for g in range(0, WPC, G)]

    # global chunk layout: for each group, for each class, for each window in
    # group, that window's class chunks (one contiguous dma_gather per
    # (group, class)).
    group_meta = []           # per group: dict with chunk/col offsets
    wmeta = [dict(schunks=[], gchunks=[]) for _ in range(WPC)]
    chunkpos = 0
    colpos = 0
    cell_slot = {}            # (w, c) -> global slot start
    for grp in groups:
        g_chunk_base = chunkpos
        g_col_base = colpos
        calls = []
        for c in range(NCLASS):
            call_chunk_start = chunkpos
            call_col_start = colpos
            cells = []
            for w in grp:
                ncw = int(chunks_wc[w, c])
                cell_slot[(w, c)] = chunkpos * P
                cells.append((chunkpos, ncw))
                chunkpos += ncw
            cn = chunkpos - call_chunk_start
            colpos += cn * P // 16
            calls.append(dict(chunk_start=call_chunk_start, nchunks=cn,
                              col_start=call_col_start, ncols=colpos - call_col_start,
                              cells=cells))
        group_meta.append(dict(chunk_base=g_chunk_base, nchunks=chunkpos - g_chunk_base,
                               col_base=g_col_base, ncols=colpos - g_col_base,
                               calls=calls, windows=list(grp)))
    TOTCHUNKS = chunkpos
    TOTCOLS = colpos

    # per-window ordered chunk lists: s-order (class-major) + matching global
    # chunk ids, and per-(w,c) count for S generation
    for w in range(WPC):
        for c in range(NCLASS):
            ncw = int(chunks_wc[w, c])
            if ncw == 0:
                continue
            base = cell_slot[(w, c)] // P
            wmeta[w]["gchunks"].extend(range(base, base + ncw))
            wmeta[w]["schunks"].append((c, ncw, base))

    # per-subcall true per-core index counts (num_idxs_reg): the SWDGE ucode
    # trims the trailing -1 pads, so padding and per-core count skew cost
    # neither descriptors nor DMA traffic.  The reg value must match the
    # trimmed count exactly (ring bookkeeping is sized from the register).
    subcall_meta = []          # (w, c, s0, sc) in program enumeration order
    for gm in group_meta:
        for c, call in enumerate(gm["calls"]):
            if call["nchunks"] == 0:
                continue
            for w, (cell_start, ncw) in zip(gm["windows"], call["cells"]):
                for s0 in range(0, ncw, 24):
                    sc = min(24, ncw - s0)
                    subcall_meta.append((w, c, s0, sc))
    NSUB = len(subcall_meta)
    gcnt = np.zeros((NCORES, NSUB), np.int32)
    for k, (w, c, s0, sc) in enumerate(subcall_meta):
        for core in range(NCORES):
            cidx = (core * WPC + w) * NCLASS + c
            gcnt[core, k] = min(max(int(cellcnt[cidx]) - s0 * P, 0), sc * P)

    # per-core slot arrays; pads gather row 0 (valid, excluded by S==0).
    # (-1 trailing-trim needs a per-core runtime num_idxs_reg, and reusing
    # registers for that races with the engine wait queue — keep it static.)
    rel_all = rel2[order]
    idx_slots = np.zeros((NCORES, TOTCHUNKS * P), np.int16)
    dst_slots = np.full((NCORES, TOTCHUNKS * P), 300.0, np.float32)
    for w in range(WPC):
        for c in range(NCLASS):
            ncw = int(chunks_wc[w, c])
            if ncw == 0:
                continue
            s0 = cell_slot[(w, c)]
            for core in range(NCORES):
                cidx = (core * WPC + w) * NCLASS + c
                cnt = int(cellcnt[cidx])
                st = int(cellstart[cidx])
                idx_slots[core, s0:s0 + cnt] = rel_all[st:st + cnt].astype(np.int16)
                dst_slots[core, s0:s0 + cnt] = (d2s[st:st + cnt] & (P - 1)).astype(np.float32)

    # wrapped int16 index tensors (per call: idx i at [i%16, i//16], tiled x8)
    idx16 = np.zeros((NCORES, 128, TOTCOLS), np.int16)
    for gm in group_meta:
        for call in gm["calls"]:
            cn = call["nchunks"]
            if cn == 0:
                continue
            s0 = call["chunk_start"] * P
            c0 = call["col_start"]
            seg = idx_slots[:, s0:s0 + cn * P]                  # [NCORES, n]
            wrapped = seg.reshape(NCORES, cn * P // 16, 16).transpose(0, 2, 1)
            idx16[:, :, c0:c0 + cn * P // 16] = np.tile(wrapped, (1, 8, 1))

    dstloc = dst_slots.reshape(NCORES, TOTCHUNKS, P).transpose(0, 2, 1)  # [NCORES,128,TOTCHUNKS]

    meta = dict(N=N, WPC=WPC, SHARD=SHARD, NPAD=NPAD,
                RS=RS.tolist(), RE=RE.tolist(), TSIZE=TSIZE,
                TOTCHUNKS=TOTCHUNKS, TOTCOLS=TOTCOLS, NSUB=NSUB,
                groups=group_meta, wmeta=wmeta,
                chunks_sig=chunks_wc.tobytes())
    return meta, deg, idx16, dstloc.astype(BF16), gcnt


# ------------------------------------------------------------- bass program


def _build_program(meta, IN_C, HID, OUT_C, debug_phase=None):
    import concourse.bacc as bacc
    import concourse.mybir as mybir
    import concourse.tile as tile

    WPC, SHARD, NPAD = meta["WPC"], meta["SHARD"], meta["NPAD"]
    RS, RE, TSIZE = meta["RS"], meta["RE"], meta["TSIZE"]
    TOTCHUNKS, TOTCOLS = meta["TOTCHUNKS"], meta["TOTCOLS"]
    KIN = IN_C // P

    nc = bacc.Bacc("TRN2", target_bir_lowering=False, debug=False,
                   num_devices=NCORES, num_swdge_queues=4)
    f32, bf16, i16, i32 = (mybir.dt.float32, mybir.dt.bfloat16,
                           mybir.dt.int16, mybir.dt.int32)

    zt_shard = nc.dram_tensor("zt_shard", [IN_C, SHARD], bf16, kind="ExternalInput").ap()
    gcnt = nc.dram_tensor("gcnt", [1, meta["NSUB"]], i32, kind="ExternalInput").ap()
    w1 = nc.dram_tensor("w1", [IN_C, HID], bf16, kind="ExternalInput").ap()
    w2 = nc.dram_tensor("w2", [HID, OUT_C], bf16, kind="ExternalInput").ap()
    idx16 = nc.dram_tensor("idx16", [128, TOTCOLS], i16, kind="ExternalInput").ap()
    dstloc = nc.dram_tensor("dstloc", [128, TOTCHUNKS], bf16, kind="ExternalInput").ap()
    dinv_col = nc.dram_tensor("dinv_col", [P, WPC], f32, kind="ExternalInput").ap()
    dinv2_col = nc.dram_tensor("dinv2_col", [P, WPC], f32, kind="ExternalInput").ap()
    sqd_row = nc.dram_tensor("sqd_row", [1, SHARD], bf16, kind="ExternalInput").ap()
    b1r = nc.dram_tensor("b1r", [1, HID], bf16, kind="ExternalInput").ap()
    b2r = nc.dram_tensor("b2r", [1, OUT_C], bf16, kind="ExternalInput").ap()
    out_shard = nc.dram_tensor("out_shard", [SHARD, OUT_C], f32, kind="ExternalOutput").ap()
    dbg = None
    if debug_phase is not None:
        dbg = nc.dram_tensor("dbg", [NPAD, HID], bf16, kind="ExternalOutput").ap()

    with tile.TileContext(nc) as tc:
        with (
            tc.tile_pool(name="dram", bufs=1, space="DRAM") as dram,
            tc.tile_pool(name="const", bufs=1) as cp,
        ):
            ag1_in = dram.tile([SHARD, HID], bf16)
            ag2_in = dram.tile([SHARD, HID], bf16)
            table1 = [dram.tile([TSIZE[r], HID], bf16, addr_space="Shared",
                                name=f"t1_{r}")
                      for r in range(NCLASS)]
            table2 = [dram.tile([TSIZE[r], HID], bf16, addr_space="Shared",
                                name=f"t2_{r}")
                      for r in range(NCLASS)]

            w1sb = cp.tile([P, KIN * HID], bf16)
            for ic in range(KIN):
                nc.sync.dma_start(w1sb[:, ic * HID:(ic + 1) * HID],
                                  w1[ic * P:(ic + 1) * P, :])
            w2sb = cp.tile([P, OUT_C], bf16)
            nc.sync.dma_start(w2sb[:], w2[:])
            dinvsb = cp.tile([P, WPC], f32)
            nc.sync.dma_start(dinvsb[:], dinv_col[:])
            dinv2sb = cp.tile([P, WPC], f32)
            nc.sync.dma_start(dinv2sb[:], dinv2_col[:])
            sqdsb = cp.tile([1, SHARD], bf16)
            nc.sync.dma_start(sqdsb[:], sqd_row[:])
            b1sb = cp.tile([1, HID], bf16)
            nc.sync.dma_start(b1sb[:], b1r[:])
            b2sb = cp.tile([1, OUT_C], bf16)
            nc.sync.dma_start(b2sb[:], b2r[:])
            gcntsb = cp.tile([1, meta["NSUB"]], i32)
            nc.sync.dma_start(gcntsb[:], gcnt[:])

            iota_i = cp.tile([P, P], i32)
            nc.gpsimd.iota(iota_i[:], pattern=[[1, P]], base=0, channel_multiplier=0)
            iota_bf = cp.tile([P, P], bf16)
            nc.vector.tensor_copy(iota_bf[:], iota_i[:])
            iotar_i = cp.tile([P, P], i32)
            nc.gpsimd.iota(iotar_i[:], pattern=[[0, P]], base=0, channel_multiplier=1)
            ident = cp.tile([P, P], bf16)
            nc.vector.tensor_tensor(out=ident[:], in0=iota_i[:], in1=iotar_i[:],
                                    op=mybir.AluOpType.is_equal)

            # ---------------- phase A: h1' = (z @ W1) * dinv  (own shard)
            with (
                tc.tile_pool(name="mmA", bufs=2) as mp,
                tc.tile_pool(name="psA", bufs=2, space="PSUM") as psA,
            ):
                ASTEP = 1024
                for t0 in range(0, SHARD, ASTEP):
                    gsz = min(ASTEP, SHARD - t0)
                    zts = []
                    for ic in range(KIN):
                        zt = mp.tile([P, gsz], bf16, tag=f"zt{ic}",
                                     padded_shape=[P, ASTEP], name=f"zt{ic}")
                        nc.sync.dma_start(
                            zt[:], zt_shard[ic * P:(ic + 1) * P, t0:t0 + gsz])
                        zts.append(zt)
                    for sub in range(gsz // P):
                        nt = t0 // P + sub
                        ps = psA.tile([P, HID], f32, name="psa")
                        for ic in range(KIN):
                            nc.tensor.matmul(
                                ps[:], lhsT=zts[ic][:, sub * P:(sub + 1) * P],
                                rhs=w1sb[:, ic * HID:(ic + 1) * HID],
                                start=(ic == 0), stop=(ic == KIN - 1))
                        hsb = mp.tile([P, HID], bf16, tag="hsb", name="hsb")
                        nc.scalar.mul(hsb[:], ps[:], dinvsb[:, nt:nt + 1])
                        nc.scalar.dma_start(ag1_in[nt * P:(nt + 1) * P, :], hsb[:])

            # chunked AllGathers: chunk r only needs window range [RS[r], RE[r])
            # of every core's phase-A output, so it fires while later ranges
            # are still being computed
            for r in range(NCLASS):
                nc.gpsimd.collective_compute(
                    "AllGather", mybir.AluOpType.bypass,
                    replica_groups=[list(range(NCORES))],
                    ins=[ag1_in[RS[r] * P:RE[r] * P, :]], outs=[table1[r][:]])

            # ---------------- aggregation layers
            def agg_layer(table, layer, selfsrc, dbg_mode=None, post_group=None):
                subidx = itertools.count()
                with (
                    tc.tile_pool(name=f"gat{layer}", bufs=2) as gp,
                    tc.tile_pool(name=f"s{layer}", bufs=3) as sp,
                    tc.tile_pool(name=f"eps{layer}", bufs=3) as ep,
                    tc.tile_pool(name=f"ps{layer}", bufs=2, space="PSUM") as pp,
                    tc.tile_pool(name=f"pso{layer}", bufs=2, space="PSUM") as po,
                ):
                    maxgch = max(gm["nchunks"] for gm in meta["groups"])
                    maxgcol = max(gm["ncols"] for gm in meta["groups"])
                    maxsch = max(len(wm["gchunks"]) for wm in meta["wmeta"])
                    for gi, gm in enumerate(meta["groups"]):
                        gch, gcol = gm["nchunks"], gm["ncols"]
                        idx_sb = gp.tile([128, gcol], i16, tag="idx",
                                         padded_shape=[128, maxgcol], name="idx_sb")
                        nc.sync.dma_start(idx_sb[:], idx16[:, gm["col_base"]:gm["col_base"] + gcol])
                        dl_sb = gp.tile([P, gch], bf16, tag="dl",
                                        padded_shape=[P, maxgch], name="dl_sb")
                        nc.sync.dma_start(dl_sb[:], dstloc[:, gm["chunk_base"]:gm["chunk_base"] + gch])
                        gbuf = gp.tile([P, gch * P], bf16, tag="gbuf",
                                       padded_shape=[P, maxgch * P], name="gbuf")
                        if gi < 2:
                            # trimmed pad slots are never DMA-written; seed the
                            # two ring buffers with finite values once so
                            # 0*garbage in the scatter matmul can't make NaN
                            nc.vector.memset(gbuf[:], 0)
                        # one subcall per (window, class) cell, ending at the
                        # cell boundary so trailing -1 pads get trimmed;
                        # round-robin across classes/queues so the sequencer
                        # never ring-blocks one queue before the other three
                        # Q7 pairs have work
                        subcalls = []
                        for c, call in enumerate(gm["calls"]):
                            if call["nchunks"] == 0:
                                continue
                            off0 = gm["chunk_base"]
                            cs = []
                            for (cell_start, ncw) in call["cells"]:
                                for s0 in range(0, ncw, 24):
                                    sc = min(24, ncw - s0)
                                    o = cell_start + s0 - off0
                                    cs.append((c, o, (call["col_start"] - gm["col_base"])
                                               + (cell_start + s0 - call["chunk_start"]) * 8,
                                               sc, next(subidx)))
                            subcalls.append(cs)
                        for tup in itertools.zip_longest(*subcalls):
                            for sub in tup:
                                if sub is None:
                                    continue
                                c, o, l0, sc, k = sub
                                cnt = sc * P
                                nc.gpsimd.dma_gather(
                                    out_ap=gbuf[:, o * P:(o + sc) * P]
                                        .rearrange("p (k f) -> p k f", f=P),
                                    in_ap=table[c][:],
                                    idxs_ap=idx_sb[:, l0:l0 + sc * 8],
                                    num_idxs=sc * P,
                                    num_idxs_reg=cnt,
                                    elem_size=HID,
                                    single_packet=(sc <= 8),
                                    queue_num=c,
                                )
                        if dbg_mode == "gather":
                            # consume gbuf: copy first window-tile to ag2_in
                            gcp = ep.tile([P, HID], bf16, tag="l1", name="gcp")
                            nc.vector.tensor_copy(gcp[:], gbuf[:, :HID])
                            nc.sync.dma_start(
                                ag2_in[gm["windows"][0] * P:(gm["windows"][0] + 1) * P, :],
                                gcp[:])
                            continue
                        for w in gm["windows"]:
                            wm = meta["wmeta"][w]
                            cw = len(wm["gchunks"])
                            # own-window table rows for the self-loop term
                            tsw = ep.tile([P, HID], bf16, tag="tsw", name="tsw")
                            nc.scalar.dma_start(tsw[:], selfsrc[w * P:(w + 1) * P, :])
                            s_sb = sp.tile([P, max(cw, 1) * P], bf16, tag="s",
                                           padded_shape=[P, maxsch * P], name="s_sb")
                            soff = 0
                            for (c, ncw, gbase) in wm["schunks"]:
                                lc0 = gbase - gm["chunk_base"]
                                in0 = (dl_sb[:, lc0:lc0 + ncw]
                                       .rearrange("p (c one) -> p c one", one=1)
                                       .to_broadcast([P, ncw, P]))
                                in1 = (iota_bf[:]
                                       .rearrange("p (one j) -> p one j", one=1)
                                       .to_broadcast([P, ncw, P]))
                                nc.vector.tensor_tensor(
                                    out=s_sb[:, soff * P:(soff + ncw) * P]
                                        .rearrange("p (c j) -> p c j", j=P),
                                    in0=in0, in1=in1,
                                    op=mybir.AluOpType.is_equal)
                                soff += ncw
                            ps = pp.tile([P, P], f32, name="ps")
                            if layer == 1:
                                use_bias = dbg_mode != "nobias"
                                if use_bias:
                                    nc.tensor.matmul(
                                        ps[:], lhsT=sqdsb[:, w * P:(w + 1) * P],
                                        rhs=b1sb[:], start=True, stop=False)
                                nc.tensor.matmul(
                                    ps[:], lhsT=ident[:], rhs=tsw[:],
                                    start=(not use_bias), stop=(cw == 0))
                                for j, gc in enumerate(wm["gchunks"]):
                                    lgc = gc - gm["chunk_base"]
                                    nc.tensor.matmul(
                                        ps[:],
                                        lhsT=s_sb[:, j * P:(j + 1) * P],
                                        rhs=gbuf[:, lgc * P:(lgc + 1) * P],
                                        start=False,
                                        stop=(j == cw - 1))
                                l2row = ep.tile([P, HID], bf16, tag="l2r", name="l2row")
                                if dbg_mode == "nobias":
                                    nc.vector.tensor_copy(l2row[:], ps[:])
                                    nc.sync.dma_start(ag2_in[w * P:(w + 1) * P, :], l2row[:])
                                    continue
                                # dinv*relu(dinv*ps) == relu(dinv^2*ps): one op
                                nc.scalar.activation(
                                    l2row[:], ps[:],
                                    mybir.ActivationFunctionType.Relu,
                                    scale=dinv2sb[:, w:w + 1])
                                nc.scalar.dma_start(ag2_in[w * P:(w + 1) * P, :], l2row[:])
                            else:
                                # transposed accumulate: ps[f, d]
                                nc.tensor.matmul(
                                    ps[:], lhsT=tsw[:], rhs=ident[:],
                                    start=True, stop=(cw == 0))
                                for j, gc in enumerate(wm["gchunks"]):
                                    lgc = gc - gm["chunk_base"]
                                    nc.tensor.matmul(
                                        ps[:],
                                        lhsT=gbuf[:, lgc * P:(lgc + 1) * P],
                                        rhs=s_sb[:, j * P:(j + 1) * P],
                                        start=False, stop=(j == cw - 1))
                                a2t = ep.tile([P, P], bf16, tag="a2t", name="a2t")
                                nc.scalar.copy(a2t[:], ps[:])
                                ops = po.tile([P, OUT_C], f32, name="ops")
                                nc.tensor.matmul(ops[:], lhsT=a2t[:], rhs=w2sb[:],
                                                 start=True, stop=False)
                                nc.tensor.matmul(ops[:], lhsT=sqdsb[:, w * P:(w + 1) * P],
                                                 rhs=b2sb[:], start=False, stop=True)
                                fsb = ep.tile([P, OUT_C], f32, tag="fout", name="fsb")
                                nc.scalar.mul(fsb[:], ops[:], dinvsb[:, w:w + 1])
                                nc.sync.dma_start(out_shard[w * P:(w + 1) * P, :], fsb[:])
                        if post_group is not None:
                            post_group(gm["windows"][0])
                    if post_group is not None:
                        post_group(WPC)

            # issue layer-2's AllGather chunk r as soon as every window of
            # range r has been written (one-group hysteresis via first_window
            # of the group *currently* finishing)
            ag2_issued = [False] * NCLASS

            def issue_ag2(first_window_of_current):
                for r in range(NCLASS):
                    if not ag2_issued[r] and RE[r] <= first_window_of_current:
                        ag2_issued[r] = True
                        nc.gpsimd.collective_compute(
                            "AllGather", mybir.AluOpType.bypass,
                            replica_groups=[list(range(NCORES))],
                            ins=[ag2_in[RS[r] * P:RE[r] * P, :]],
                            outs=[table2[r][:]])

            if debug_phase == "A":
                for r in range(NCLASS):
                    nc.sync.dma_start(
                        dbg[sum(TSIZE[:r]):sum(TSIZE[:r + 1]), :], table1[r][:])
            else:
                agg_layer(table1, 1, ag1_in,
                          dbg_mode=debug_phase if debug_phase in ("gather", "nobias") else None,
                          post_group=None if debug_phase in ("C1", "gather", "nobias")
                          else issue_ag2)
                if debug_phase in ("C1", "gather", "nobias"):
                    nc.sync.dma_start(dbg[:SHARD, :], ag2_in[:])
                else:
                    agg_layer(table2, 2, ag2_in)

    nc.compile()
    return nc


# ----------------------------------------------------------------- entry


def _prepare_and_build(z, edge_index, W1, b1, W2, b2):
    N, IN_C = z.shape
    HID = W1.shape[1]
    OUT_C = W2.shape[1]
    meta, deg, idx16, dstloc, gcnt = _plan(edge_index, N)
    WPC, SHARD, NPAD = meta["WPC"], meta["SHARD"], meta["NPAD"]

    dinv = (1.0 / np.sqrt(deg)).astype(np.float32)
    dinv_pad = np.zeros(NPAD, np.float32)
    dinv_pad[:N] = dinv
    sqd_pad = np.zeros(NPAD, np.float32)
    sqd_pad[:N] = np.sqrt(deg).astype(np.float32)

    zpad = np.zeros((NPAD, IN_C), BF16)
    zpad[:N] = z.astype(BF16)

    w1b = np.ascontiguousarray(W1.astype(BF16))
    w2b = np.ascontiguousarray(W2.astype(BF16))
    b1b = np.ascontiguousarray(b1.reshape(1, HID).astype(BF16))
    b2b = np.ascontiguousarray(b2.reshape(1, OUT_C).astype(BF16))

    in_maps = []
    for c in range(NCORES):
        sl = slice(c * SHARD, (c + 1) * SHARD)
        in_maps.append({
            "zt_shard": np.ascontiguousarray(zpad[sl].T),
            "gcnt": np.ascontiguousarray(gcnt[c:c + 1]),
            "w1": w1b, "w2": w2b,
            "idx16": np.ascontiguousarray(idx16[c]),
            "dstloc": np.ascontiguousarray(dstloc[c]),
            "dinv_col": np.ascontiguousarray(dinv_pad[sl].reshape(WPC, P).T),
            "dinv2_col": np.ascontiguousarray((dinv_pad[sl] ** 2).reshape(WPC, P).T),
            "sqd_row": np.ascontiguousarray(sqd_pad[sl].reshape(1, SHARD).astype(BF16)),
            "b1r": b1b, "b2r": b2b,
        })

    cache_key = (N, IN_C, HID, OUT_C, meta["TOTCHUNKS"], hash(meta["chunks_sig"]))
    if cache_key in _PROGRAM_CACHE:
        nc = _PROGRAM_CACHE[cache_key]
    else:
        nc = _build_program(meta, IN_C, HID, OUT_C)
        _PROGRAM_CACHE[cache_key] = nc
    return nc, in_maps, meta


def _run(inputs, trace=False, trace_kwargs=None):
    from concourse.bass_utils import run_bass_kernel_spmd

    z = np.asarray(inputs["z"])
    edge_index = np.asarray(inputs["edge_index"])
    W1 = np.asarray(inputs["W1"])
    b1 = np.asarray(inputs["b1"])
    W2 = np.asarray(inputs["W2"])
    b2 = np.asarray(inputs["b2"])

    nc, in_maps, meta = _prepare_and_build(z, edge_index, W1, b1, W2, b2)
    res = run_bass_kernel_spmd(
        nc, in_maps, core_ids=list(range(NCORES)),
        trace=trace, **(trace_kwargs or {}))
    N = meta["N"]
    out = np.concatenate([r["out_shard"] for r in res.results], axis=0)[:N]
    return np.ascontiguousarray(out.astype(np.float32)), res


def kernel(**inputs):
    out, _ = _run(inputs, trace=False)
    return out



# revision 58
# speedup vs baseline: 1.0054x; 1.0054x over previous
"""Two-layer GCN (PyG GCNConv x2 with relu between) on 8 Trainium2 NeuronCores.

Math (per layer, A' = D^-1/2 (A + I) D^-1/2):
    h  = relu(A' (z @ W1) + b1)
    out = A' (h @ W2) + b2  ==  (A' h) @ W2 + b2      (aggregation commutes with the
                                                       feature-space linear map)
Both layers therefore aggregate 128-wide features only.

Distribution: nodes (and dst-partitioned edges) sharded across 8 cores;
weights replicated; per-layer AllGather of the (dinv-scaled) feature table in
bf16; per-core gather of source rows via bulk SWDGE dma_gather; segment-sum
realized as one-hot matmuls accumulating in PSUM.

The Bass program is specialized to the actual graph: per-(window, class)
chunk counts are compile-time constants derived from edge_index.
"""

import itertools

import numpy as np
import ml_dtypes

P = 128
NCORES = 8
NCLASS = 4          # src-range classes so relative gather indices fit int16
G = 4               # dst windows per gather group

BF16 = ml_dtypes.bfloat16

_PROGRAM_CACHE = {}


# ----------------------------------------------------------------- host prep


def _plan(edge_index, N):
    """Sort/partition edges; all compile-time metadata + per-core slot arrays.

    src "classes" are window-ranges: class r covers windows [RS[r], RE[r]) of
    every core.  The per-class table chunk (one AllGather each) is laid out
    [core, window-in-range, 128] so a chunk's AllGather can fire as soon as
    every core has produced that window range.
    """
    WPC = -(-N // (NCORES * P))            # windows per core
    SHARD = WPC * P
    NPAD = NCORES * SHARD

    # make the LAST range smallest: its AllGather chunk is the one whose
    # latency is exposed between phases
    last = max(WPC // NCLASS - 6, 1)
    rest = WPC - last
    b3, r3 = divmod(rest, NCLASS - 1)
    RW = np.array([b3 + (1 if r < r3 else 0) for r in range(NCLASS - 1)] + [last])
    RS = np.concatenate([[0], np.cumsum(RW)])[:NCLASS]
    RE = RS + RW
    TSIZE = (NCORES * RW * P).tolist()
    assert max(NCORES * RW * P) <= 32767
    range_of = np.repeat(np.arange(NCLASS), RW)          # [WPC] -> class

    src = np.asarray(edge_index[0], dtype=np.int64)
    dst = np.asarray(edge_index[1], dtype=np.int64)
    deg = np.bincount(dst, minlength=N).astype(np.float64) + 1.0

    # self-loops are added on-chip via an identity matmul over the local
    # window tile; only real edges go through the gather
    s2 = src
    d2 = dst

    def relidx(s):
        k = s // SHARD
        wloc = (s % SHARD) >> 7
        r = range_of[wloc]
        return (k * RW[r] + (wloc - RS[r])) * P + (s & (P - 1)), r

    win = d2 >> 7
    rel2, cls = relidx(s2)
    key = win * NCLASS + cls
    # secondary sort by table row: each SDMA engine then walks ascending
    # addresses within a gather call (HBM locality)
    order = np.lexsort((rel2, key))
    d2s = d2[order]

    NW = NPAD // P
    cellcnt = np.bincount(key, minlength=NW * NCLASS)
    cellstart = np.concatenate([[0], np.cumsum(cellcnt)]).astype(np.int64)
    counts_core = cellcnt.reshape(NCORES, WPC, NCLASS)
    chunks_wc = -(-counts_core.max(axis=0) // P)      # [WPC, NCLASS]

    groups = [list(range(g, min(g + G, WPC))) for g in range(0, WPC, G)]

    # global chunk layout: for each group, for each class, for each window in
    # group, that window's class chunks (one contiguous dma_gather per
    # (group, class)).
    group_meta = []           # per group: dict with chunk/col offsets
    wmeta = [dict(schunks=[], gchunks=[]) for _ in range(WPC)]
    chunkpos = 0
    colpos = 0
    cell_slot = {}            # (w, c) -> global slot start
    for grp in groups:
        g_chunk_base = chunkpos
        g_col_base = colpos
        calls = []
        for c in range(NCLASS):
            call_chunk_start = chunkpos
            call_col_start = colpos
            cells = []
            for w in grp:
                ncw = int(chunks_wc[w, c])
                cell_slot[(w, c)] = chunkpos * P
                cells.append((chunkpos, ncw))
                chunkpos += ncw
            cn = chunkpos - call_chunk_start
            colpos += cn * P // 16
            calls.append(dict(chunk_start=call_chunk_start, nchunks=cn,
                              col_start=call_col_start, ncols=colpos - call_col_start,
                              cells=cells))
        group_meta.append(dict(chunk_base=g_chunk_base, nchunks=chunkpos - g_chunk_base,
                               col_base=g_col_base, ncols=colpos - g_col_base,
                               calls=calls, windows=list(grp)))
    TOTCHUNKS = chunkpos
    TOTCOLS = colpos

    # per-window ordered chunk lists: s-order (class-major) + matching global
    # chunk ids, and per-(w,c) count for S generation
    for w in range(WPC):
        for c in range(NCLASS):
            ncw = int(chunks_wc[w, c])
            if ncw == 0:
                continue
            base = cell_slot[(w, c)] // P
            wmeta[w]["gchunks"].extend(range(base, base + ncw))
            wmeta[w]["schunks"].append((c, ncw, base))

    # per-subcall true per-core index counts (num_idxs_reg): the SWDGE ucode
    # trims the trailing -1 pads, so padding and per-core count skew cost
    # neither descriptors nor DMA traffic.  The reg value must match the
    # trimmed count exactly (ring bookkeeping is sized from the register).
    subcall_meta = []          # (w, c, s0, sc) in program enumeration order
    for gm in group_meta:
        for c, call in enumerate(gm["calls"]):
            if call["nchunks"] == 0:
                continue
            for w, (cell_start, ncw) in zip(gm["windows"], call["cells"]):
                for s0 in range(0, ncw, 24):
                    sc = min(24, ncw - s0)
                    subcall_meta.append((w, c, s0, sc))
    NSUB = len(subcall_meta)
    gcnt = np.zeros((NCORES, NSUB), np.int32)
    for k, (w, c, s0, sc) in enumerate(subcall_meta):
        for core in range(NCORES):
            cidx = (core * WPC + w) * NCLASS + c
            gcnt[core, k] = min(max(int(cellcnt[cidx]) - s0 * P, 0), sc * P)

    # per-core slot arrays; pads gather row 0 (valid, excluded by S==0).
    # (-1 trailing-trim needs a per-core runtime num_idxs_reg, and reusing
    # registers for that races with the engine wait queue — keep it static.)
    rel_all = rel2[order]
    idx_slots = np.zeros((NCORES, TOTCHUNKS * P), np.int16)
    dst_slots = np.full((NCORES, TOTCHUNKS * P), 300.0, np.float32)
    for w in range(WPC):
        for c in range(NCLASS):
            ncw = int(chunks_wc[w, c])
            if ncw == 0:
                continue
            s0 = cell_slot[(w, c)]
            for core in range(NCORES):
                cidx = (core * WPC + w) * NCLASS + c
                cnt = int(cellcnt[cidx])
                st = int(cellstart[cidx])
                idx_slots[core, s0:s0 + cnt] = rel_all[st:st + cnt].astype(np.int16)
                dst_slots[core, s0:s0 + cnt] = (d2s[st:st + cnt] & (P - 1)).astype(np.float32)

    # wrapped int16 index tensors (per call: idx i at [i%16, i//16], tiled x8)
    idx16 = np.zeros((NCORES, 128, TOTCOLS), np.int16)
    for gm in group_meta:
        for call in gm["calls"]:
            cn = call["nchunks"]
            if cn == 0:
                continue
            s0 = call["chunk_start"] * P
            c0 = call["col_start"]
            seg = idx_slots[:, s0:s0 + cn * P]                  # [NCORES, n]
            wrapped = seg.reshape(NCORES, cn * P // 16, 16).transpose(0, 2, 1)
            idx16[:, :, c0:c0 + cn * P // 16] = np.tile(wrapped, (1, 8, 1))

    dstloc = dst_slots.reshape(NCORES, TOTCHUNKS, P).transpose(0, 2, 1)  # [NCORES,128,TOTCHUNKS]

    meta = dict(N=N, WPC=WPC, SHARD=SHARD, NPAD=NPAD,
                RS=RS.tolist(), RE=RE.tolist(), TSIZE=TSIZE,
                TOTCHUNKS=TOTCHUNKS, TOTCOLS=TOTCOLS, NSUB=NSUB,
                groups=group_meta, wmeta=wmeta,
                chunks_sig=chunks_wc.tobytes())
    return meta, deg, idx16, dstloc.astype(BF16), gcnt


# ------------------------------------------------------------- bass program


def _build_program(meta, IN_C, HID, OUT_C, debug_phase=None):
    import concourse.bacc as bacc
    import concourse.mybir as mybir
    import concourse.tile as tile

    WPC, SHARD, NPAD = meta["WPC"], meta["SHARD"], meta["NPAD"]
    RS, RE, TSIZE = meta["RS"], meta["RE"], meta["TSIZE"]
    TOTCHUNKS, TOTCOLS = meta["TOTCHUNKS"], meta["TOTCOLS"]
    KIN = IN_C // P

    nc = bacc.Bacc("TRN2", target_bir_lowering=False, debug=False,
                   num_devices=NCORES, num_swdge_queues=4)
    f32, bf16, i16, i32 = (mybir.dt.float32, mybir.dt.bfloat16,
                           mybir.dt.int16, mybir.dt.int32)

    zt_shard = nc.dram_tensor("zt_shard", [IN_C, SHARD], bf16, kind="ExternalInput").ap()
    gcnt = nc.dram_tensor("gcnt", [1, meta["NSUB"]], i32, kind="ExternalInput").ap()
    w1 = nc.dram_tensor("w1", [IN_C, HID], bf16, kind="ExternalInput").ap()
    w2 = nc.dram_tensor("w2", [HID, OUT_C], bf16, kind="ExternalInput").ap()
    idx16 = nc.dram_tensor("idx16", [128, TOTCOLS], i16, kind="ExternalInput").ap()
    dstloc = nc.dram_tensor("dstloc", [128, TOTCHUNKS], bf16, kind="ExternalInput").ap()
    dinv_col = nc.dram_tensor("dinv_col", [P, WPC], f32, kind="ExternalInput").ap()
    dinv2_col = nc.dram_tensor("dinv2_col", [P, WPC], f32, kind="ExternalInput").ap()
    sqd_row = nc.dram_tensor("sqd_row", [1, SHARD], bf16, kind="ExternalInput").ap()
    b1r = nc.dram_tensor("b1r", [1, HID], bf16, kind="ExternalInput").ap()
    b2r = nc.dram_tensor("b2r", [1, OUT_C], bf16, kind="ExternalInput").ap()
    out_shard = nc.dram_tensor("out_shard", [SHARD, OUT_C], f32, kind="ExternalOutput").ap()
    dbg = None
    if debug_phase is not None:
        dbg = nc.dram_tensor("dbg", [NPAD, HID], bf16, kind="ExternalOutput").ap()

    with tile.TileContext(nc) as tc:
        with (
            tc.tile_pool(name="dram", bufs=1, space="DRAM") as dram,
            tc.tile_pool(name="const", bufs=1) as cp,
        ):
            ag1_in = dram.tile([SHARD, HID], bf16)
            ag2_in = dram.tile([SHARD, HID], bf16)
            table1 = [dram.tile([TSIZE[r], HID], bf16, addr_space="Shared",
                                name=f"t1_{r}")
                      for r in range(NCLASS)]
            table2 = [dram.tile([TSIZE[r], HID], bf16, addr_space="Shared",
                                name=f"t2_{r}")
                      for r in range(NCLASS)]

            w1sb = cp.tile([P, KIN * HID], bf16)
            for ic in range(KIN):
                nc.sync.dma_start(w1sb[:, ic * HID:(ic + 1) * HID],
                                  w1[ic * P:(ic + 1) * P, :])
            w2sb = cp.tile([P, OUT_C], bf16)
            nc.sync.dma_start(w2sb[:], w2[:])
            dinvsb = cp.tile([P, WPC], f32)
            nc.sync.dma_start(dinvsb[:], dinv_col[:])
            dinv2sb = cp.tile([P, WPC], f32)
            nc.sync.dma_start(dinv2sb[:], dinv2_col[:])
            sqdsb = cp.tile([1, SHARD], bf16)
            nc.sync.dma_start(sqdsb[:], sqd_row[:])
            b1sb = cp.tile([1, HID], bf16)
            nc.sync.dma_start(b1sb[:], b1r[:])
            b2sb = cp.tile([1, OUT_C], bf16)
            nc.sync.dma_start(b2sb[:], b2r[:])
            gcntsb = cp.tile([1, meta["NSUB"]], i32)
            nc.sync.dma_start(gcntsb[:], gcnt[:])

            iota_i = cp.tile([P, P], i32)
            nc.gpsimd.iota(iota_i[:], pattern=[[1, P]], base=0, channel_multiplier=0)
            iota_bf = cp.tile([P, P], bf16)
            nc.vector.tensor_copy(iota_bf[:], iota_i[:])
            iotar_i = cp.tile([P, P], i32)
            nc.gpsimd.iota(iotar_i[:], pattern=[[0, P]], base=0, channel_multiplier=1)
            ident = cp.tile([P, P], bf16)
            nc.vector.tensor_tensor(out=ident[:], in0=iota_i[:], in1=iotar_i[:],
                                    op=mybir.AluOpType.is_equal)

            # ---------------- phase A: h1' = (z @ W1) * dinv  (own shard)
            with (
                tc.tile_pool(name="mmA", bufs=2) as mp,
                tc.tile_pool(name="psA", bufs=2, space="PSUM") as psA,
            ):
                ASTEP = 1024
                for t0 in range(0, SHARD, ASTEP):
                    gsz = min(ASTEP, SHARD - t0)
                    zts = []
                    for ic in range(KIN):
                        zt = mp.tile([P, gsz], bf16, tag=f"zt{ic}",
                                     padded_shape=[P, ASTEP], name=f"zt{ic}")
                        nc.sync.dma_start(
                            zt[:], zt_shard[ic * P:(ic + 1) * P, t0:t0 + gsz])
                        zts.append(zt)
                    for sub in range(gsz // P):
                        nt = t0 // P + sub
                        ps = psA.tile([P, HID], f32, name="psa")
                        for ic in range(KIN):
                            nc.tensor.matmul(
                                ps[:], lhsT=zts[ic][:, sub * P:(sub + 1) * P],
                                rhs=w1sb[:, ic * HID:(ic + 1) * HID],
                                start=(ic == 0), stop=(ic == KIN - 1))
                        hsb = mp.tile([P, HID], bf16, tag="hsb", name="hsb")
                        nc.scalar.mul(hsb[:], ps[:], dinvsb[:, nt:nt + 1])
                        nc.scalar.dma_start(ag1_in[nt * P:(nt + 1) * P, :], hsb[:])

            # chunked AllGathers: chunk r only needs window range [RS[r], RE[r])
            # of every core's phase-A output, so it fires while later ranges
            # are still being computed
            for r in range(NCLASS):
                nc.gpsimd.collective_compute(
                    "AllGather", mybir.AluOpType.bypass,
                    replica_groups=[list(range(NCORES))],
                    ins=[ag1_in[RS[r] * P:RE[r] * P, :]], outs=[table1[r][:]])

            # ---------------- aggregation layers
            def agg_layer(table, layer, selfsrc, dbg_mode=None, post_group=None):
                subidx = itertools.count()
                with (
                    tc.tile_pool(name=f"gat{layer}", bufs=2) as gp,
                    tc.tile_pool(name=f"s{layer}", bufs=3) as sp,
                    tc.tile_pool(name=f"eps{layer}", bufs=3) as ep,
                    tc.tile_pool(name=f"ps{layer}", bufs=2, space="PSUM") as pp,
                    tc.tile_pool(name=f"pso{layer}", bufs=2, space="PSUM") as po,
                ):
                    maxgch = max(gm["nchunks"] for gm in meta["groups"])
                    maxgcol = max(gm["ncols"] for gm in meta["groups"])
                    maxsch = max(len(wm["gchunks"]) for wm in meta["wmeta"])
                    for gi, gm in enumerate(meta["groups"]):
                        gch, gcol = gm["nchunks"], gm["ncols"]
                        idx_sb = gp.tile([128, gcol], i16, tag="idx",
                                         padded_shape=[128, maxgcol], name="idx_sb")
                        nc.sync.dma_start(idx_sb[:], idx16[:, gm["col_base"]:gm["col_base"] + gcol])
                        dl_sb = gp.tile([P, gch], bf16, tag="dl",
                                        padded_shape=[P, maxgch], name="dl_sb")
                        nc.sync.dma_start(dl_sb[:], dstloc[:, gm["chunk_base"]:gm["chunk_base"] + gch])
                        gbuf = gp.tile([P, gch * P], bf16, tag="gbuf",
                                       padded_shape=[P, maxgch * P], name="gbuf")
                        if gi < 2:
                            # trimmed pad slots are never DMA-written; seed the
                            # two ring buffers with finite values once so
                            # 0*garbage in the scatter matmul can't make NaN
                            nc.vector.memset(gbuf[:], 0)
                        # one subcall per (window, class) cell, ending at the
                        # cell boundary so trailing -1 pads get trimmed;
                        # round-robin across classes/queues so the sequencer
                        # never ring-blocks one queue before the other three
                        # Q7 pairs have work
                        subcalls = []
                        for c, call in enumerate(gm["calls"]):
                            if call["nchunks"] == 0:
                                continue
                            off0 = gm["chunk_base"]
                            SUB = 15
                            cn = call["nchunks"]
                            nsub = -(-cn // SUB)
                            per = -(-cn // nsub)
                            off = call["chunk_start"] - gm["chunk_base"]
                            loc0 = call["col_start"] - gm["col_base"]
                            cs = [(c, off + s0, loc0 + s0 * 8,
                                   min(per, cn - s0), next(subidx))
                                  for s0 in range(0, cn, per)]
                            subcalls.append(cs)
                        for tup in itertools.zip_longest(*subcalls):
                            for sub in tup:
                                if sub is None:
                                    continue
                                c, o, l0, sc, k = sub
                                cnt = sc * P
                                nc.gpsimd.dma_gather(
                                    out_ap=gbuf[:, o * P:(o + sc) * P]
                                        .rearrange("p (k f) -> p k f", f=P),
                                    in_ap=table[c][:],
                                    idxs_ap=idx_sb[:, l0:l0 + sc * 8],
                                    num_idxs=sc * P,
                                    num_idxs_reg=cnt,
                                    elem_size=HID,
                                    single_packet=(sc <= 8),
                                    queue_num=c,
                                )
                        if dbg_mode == "gather":
                            # consume gbuf: copy first window-tile to ag2_in
                            gcp = ep.tile([P, HID], bf16, tag="l1", name="gcp")
                            nc.vector.tensor_copy(gcp[:], gbuf[:, :HID])
                            nc.sync.dma_start(
                                ag2_in[gm["windows"][0] * P:(gm["windows"][0] + 1) * P, :],
                                gcp[:])
                            continue
                        for w in gm["windows"]:
                            wm = meta["wmeta"][w]
                            cw = len(wm["gchunks"])
                            # own-window table rows for the self-loop term
                            tsw = ep.tile([P, HID], bf16, tag="tsw", name="tsw")
                            nc.scalar.dma_start(tsw[:], selfsrc[w * P:(w + 1) * P, :])
                            s_sb = sp.tile([P, max(cw, 1) * P], bf16, tag="s",
                                           padded_shape=[P, maxsch * P], name="s_sb")
                            soff = 0
                            for (c, ncw, gbase) in wm["schunks"]:
                                lc0 = gbase - gm["chunk_base"]
                                in0 = (dl_sb[:, lc0:lc0 + ncw]
                                       .rearrange("p (c one) -> p c one", one=1)
                                       .to_broadcast([P, ncw, P]))
                                in1 = (iota_bf[:]
                                       .rearrange("p (one j) -> p one j", one=1)
                                       .to_broadcast([P, ncw, P]))
                                nc.vector.tensor_tensor(
                                    out=s_sb[:, soff * P:(soff + ncw) * P]
                                        .rearrange("p (c j) -> p c j", j=P),
                                    in0=in0, in1=in1,
                                    op=mybir.AluOpType.is_equal)
                                soff += ncw
                            ps = pp.tile([P, P], f32, name="ps")
                            if layer == 1:
                                use_bias = dbg_mode != "nobias"
                                if use_bias:
                                    nc.tensor.matmul(
                                        ps[:], lhsT=sqdsb[:, w * P:(w + 1) * P],
                                        rhs=b1sb[:], start=True, stop=False)
                                nc.tensor.matmul(
                                    ps[:], lhsT=ident[:], rhs=tsw[:],
                                    start=(not use_bias), stop=(cw == 0))
                                for j, gc in enumerate(wm["gchunks"]):
                                    lgc = gc - gm["chunk_base"]
                                    nc.tensor.matmul(
                                        ps[:],
                                        lhsT=s_sb[:, j * P:(j + 1) * P],
                                        rhs=gbuf[:, lgc * P:(lgc + 1) * P],
                                        start=False,
                                        stop=(j == cw - 1))
                                l2row = ep.tile([P, HID], bf16, tag="l2r", name="l2row")
                                if dbg_mode == "nobias":
                                    nc.vector.tensor_copy(l2row[:], ps[:])
                                    nc.sync.dma_start(ag2_in[w * P:(w + 1) * P, :], l2row[:])
                                    continue
                                # dinv*relu(dinv*ps) == relu(dinv^2*ps): one op
                                nc.scalar.activation(
                                    l2row[:], ps[:],
                                    mybir.ActivationFunctionType.Relu,
                                    scale=dinv2sb[:, w:w + 1])
                                nc.scalar.dma_start(ag2_in[w * P:(w + 1) * P, :], l2row[:])
                            else:
                                # transposed accumulate: ps[f, d]
                                nc.tensor.matmul(
                                    ps[:], lhsT=tsw[:], rhs=ident[:],
                                    start=True, stop=(cw == 0))
                                for j, gc in enumerate(wm["gchunks"]):
                                    lgc = gc - gm["chunk_base"]
                                    nc.tensor.matmul(
                                        ps[:],
                                        lhsT=gbuf[:, lgc * P:(lgc + 1) * P],
                                        rhs=s_sb[:, j * P:(j + 1) * P],
                                        start=False, stop=(j == cw - 1))
                                a2t = ep.tile([P, P], bf16, tag="a2t", name="a2t")
                                nc.scalar.copy(a2t[:], ps[:])
                                ops = po.tile([P, OUT_C], f32, name="ops")
                                nc.tensor.matmul(ops[:], lhsT=a2t[:], rhs=w2sb[:],
                                                 start=True, stop=False)
                                nc.tensor.matmul(ops[:], lhsT=sqdsb[:, w * P:(w + 1) * P],
                                                 rhs=b2sb[:], start=False, stop=True)
                                fsb = ep.tile([P, OUT_C], f32, tag="fout", name="fsb")
                                nc.scalar.mul(fsb[:], ops[:], dinvsb[:, w:w + 1])
                                nc.sync.dma_start(out_shard[w * P:(w + 1) * P, :], fsb[:])
                        if post_group is not None:
                            post_group(gm["windows"][0])
                    if post_group is not None:
                        post_group(WPC)

            # issue layer-2's AllGather chunk r as soon as every window of
            # range r has been written (one-group hysteresis via first_window
            # of the group *currently* finishing)
            ag2_issued = [False] * NCLASS

            def issue_ag2(first_window_of_current):
                for r in range(NCLASS):
                    if not ag2_issued[r] and RE[r] <= first_window_of_current:
                        ag2_issued[r] = True
                        nc.gpsimd.collective_compute(
                            "AllGather", mybir.AluOpType.bypass,
                            replica_groups=[list(range(NCORES))],
                            ins=[ag2_in[RS[r] * P:RE[r] * P, :]],
                            outs=[table2[r][:]])

            if debug_phase == "A":
                for r in range(NCLASS):
                    nc.sync.dma_start(
                        dbg[sum(TSIZE[:r]):sum(TSIZE[:r + 1]), :], table1[r][:])
            else:
                agg_layer(table1, 1, ag1_in,
                          dbg_mode=debug_phase if debug_phase in ("gather", "nobias") else None,
                          post_group=None if debug_phase in ("C1", "gather", "nobias")
                          else issue_ag2)
                if debug_phase in ("C1", "gather", "nobias"):
                    nc.sync.dma_start(dbg[:SHARD, :], ag2_in[:])
                else:
                    agg_layer(table2, 2, ag2_in)

    nc.compile()
    return nc


# ----------------------------------------------------------------- entry


def _prepare_and_build(z, edge_index, W1, b1, W2, b2):
    N, IN_C = z.shape
    HID = W1.shape[1]
    OUT_C = W2.shape[1]
    meta, deg, idx16, dstloc, gcnt = _plan(edge_index, N)
    WPC, SHARD, NPAD = meta["WPC"], meta["SHARD"], meta["NPAD"]

    dinv = (1.0 / np.sqrt(deg)).astype(np.float32)
    dinv_pad = np.zeros(NPAD, np.float32)
    dinv_pad[:N] = dinv
    sqd_pad = np.zeros(NPAD, np.float32)
    sqd_pad[:N] = np.sqrt(deg).astype(np.float32)

    zpad = np.zeros((NPAD, IN_C), BF16)
    zpad[:N] = z.astype(BF16)

    w1b = np.ascontiguousarray(W1.astype(BF16))
    w2b = np.ascontiguousarray(W2.astype(BF16))
    b1b = np.ascontiguousarray(b1.reshape(1, HID).astype(BF16))
    b2b = np.ascontiguousarray(b2.reshape(1, OUT_C).astype(BF16))

    in_maps = []
    for c in range(NCORES):
        sl = slice(c * SHARD, (c + 1) * SHARD)
        in_maps.append({
            "zt_shard": np.ascontiguousarray(zpad[sl].T),
            "gcnt": np.ascontiguousarray(gcnt[c:c + 1]),
            "w1": w1b, "w2": w2b,
            "idx16": np.ascontiguousarray(idx16[c]),
            "dstloc": np.ascontiguousarray(dstloc[c]),
            "dinv_col": np.ascontiguousarray(dinv_pad[sl].reshape(WPC, P).T),
            "dinv2_col": np.ascontiguousarray((dinv_pad[sl] ** 2).reshape(WPC, P).T),
            "sqd_row": np.ascontiguousarray(sqd_pad[sl].reshape(1, SHARD).astype(BF16)),
            "b1r": b1b, "b2r": b2b,
        })

    cache_key = (N, IN_C, HID, OUT_C, meta["TOTCHUNKS"], hash(meta["chunks_sig"]))
    if cache_key in _PROGRAM_CACHE:
        nc = _PROGRAM_CACHE[cache_key]
    else:
        nc = _build_program(meta, IN_C, HID, OUT_C)
        _PROGRAM_CACHE[cache_key] = nc
    return nc, in_maps, meta


def _run(inputs, trace=False, trace_kwargs=None):
    from concourse.bass_utils import run_bass_kernel_spmd

    z = np.asarray(inputs["z"])
    edge_index = np.asarray(inputs["edge_index"])
    W1 = np.asarray(inputs["W1"])
    b1 = np.asarray(inputs["b1"])
    W2 = np.asarray(inputs["W2"])
    b2 = np.asarray(inputs["b2"])

    nc, in_maps, meta = _prepare_and_build(z, edge_index, W1, b1, W2, b2)
    res = run_bass_kernel_spmd(
        nc, in_maps, core_ids=list(range(NCORES)),
        trace=trace, **(trace_kwargs or {}))
    N = meta["N"]
    out = np.concatenate([r["out_shard"] for r in res.results], axis=0)[:N]
    return np.ascontiguousarray(out.astype(np.float32)), res


def kernel(**inputs):
    out, _ = _run(inputs, trace=False)
    return out



# revision 66
# speedup vs baseline: 1.2419x; 1.2352x over previous
"""Two-layer GCN (PyG GCNConv x2 with relu between) on 8 Trainium2 NeuronCores.

Math (per layer, A' = D^-1/2 (A + I) D^-1/2):
    h  = relu(A' (z @ W1) + b1)
    out = A' (h @ W2) + b2  ==  (A' h) @ W2 + b2      (aggregation commutes with the
                                                       feature-space linear map)
Both layers therefore aggregate 128-wide features only.

Distribution: nodes (and dst-partitioned edges) sharded across 8 cores;
weights replicated; per-layer AllGather of the (dinv-scaled) feature table in
bf16; per-core gather of source rows via bulk SWDGE dma_gather; segment-sum
realized as one-hot matmuls accumulating in PSUM.

The Bass program is specialized to the actual graph: per-(window, class)
chunk counts are compile-time constants derived from edge_index.
"""

import itertools

import numpy as np
import ml_dtypes

P = 128
NCORES = 8
NCLASS = 4          # src-range classes so relative gather indices fit int16
G = 4               # dst windows per gather group

BF16 = ml_dtypes.bfloat16

_PROGRAM_CACHE = {}


# ----------------------------------------------------------------- host prep


def _plan(edge_index, N):
    """Sort/partition edges; all compile-time metadata + per-core slot arrays.

    src "classes" are window-ranges: class r covers windows [RS[r], RE[r]) of
    every core.  The per-class table chunk (one AllGather each) is laid out
    [core, window-in-range, 128] so a chunk's AllGather can fire as soon as
    every core has produced that window range.
    """
    WPC = -(-N // (NCORES * P))            # windows per core
    SHARD = WPC * P
    NPAD = NCORES * SHARD

    # make the LAST range smallest: its AllGather chunk is the one whose
    # latency is exposed between phases
    last = max(WPC // NCLASS - 6, 1)
    rest = WPC - last
    b3, r3 = divmod(rest, NCLASS - 1)
    RW = np.array([b3 + (1 if r < r3 else 0) for r in range(NCLASS - 1)] + [last])
    RS = np.concatenate([[0], np.cumsum(RW)])[:NCLASS]
    RE = RS + RW
    TSIZE = (NCORES * RW * P).tolist()
    assert max(NCORES * RW * P) <= 32767
    range_of = np.repeat(np.arange(NCLASS), RW)          # [WPC] -> class

    src = np.asarray(edge_index[0], dtype=np.int64)
    dst = np.asarray(edge_index[1], dtype=np.int64)
    deg = np.bincount(dst, minlength=N).astype(np.float64) + 1.0

    # self-loops are added on-chip via an identity matmul over the local
    # window tile; only real edges go through the gather
    s2 = src
    d2 = dst

    def relidx(s):
        k = s // SHARD
        wloc = (s % SHARD) >> 7
        r = range_of[wloc]
        return (k * RW[r] + (wloc - RS[r])) * P + (s & (P - 1)), r

    win = d2 >> 7
    rel2, cls = relidx(s2)
    key = win * NCLASS + cls
    # secondary sort by table row: each SDMA engine then walks ascending
    # addresses within a gather call (HBM locality)
    order = np.lexsort((rel2, key))
    d2s = d2[order]

    NW = NPAD // P
    cellcnt = np.bincount(key, minlength=NW * NCLASS)
    cellstart = np.concatenate([[0], np.cumsum(cellcnt)]).astype(np.int64)
    counts_core = cellcnt.reshape(NCORES, WPC, NCLASS)
    chunks_wc = -(-counts_core.max(axis=0) // P)      # [WPC, NCLASS]

    groups = [list(range(g, min(g + G, WPC))) for g in range(0, WPC, G)]

    # global chunk layout: for each group, for each class, for each window in
    # group, that window's class chunks (one contiguous dma_gather per
    # (group, class)).
    group_meta = []           # per group: dict with chunk/col offsets
    wmeta = [dict(schunks=[], gchunks=[]) for _ in range(WPC)]
    chunkpos = 0
    colpos = 0
    cell_slot = {}            # (w, c) -> global slot start
    for grp in groups:
        g_chunk_base = chunkpos
        g_col_base = colpos
        calls = []
        for c in range(NCLASS):
            call_chunk_start = chunkpos
            call_col_start = colpos
            cells = []
            for w in grp:
                ncw = int(chunks_wc[w, c])
                cell_slot[(w, c)] = chunkpos * P
                cells.append((chunkpos, ncw))
                chunkpos += ncw
            cn = chunkpos - call_chunk_start
            colpos += cn * P // 16
            calls.append(dict(chunk_start=call_chunk_start, nchunks=cn,
                              col_start=call_col_start, ncols=colpos - call_col_start,
                              cells=cells))
        group_meta.append(dict(chunk_base=g_chunk_base, nchunks=chunkpos - g_chunk_base,
                               col_base=g_col_base, ncols=colpos - g_col_base,
                               calls=calls, windows=list(grp)))
    TOTCHUNKS = chunkpos
    TOTCOLS = colpos

    # per-window ordered chunk lists: s-order (class-major) + matching global
    # chunk ids, and per-(w,c) count for S generation
    for w in range(WPC):
        for c in range(NCLASS):
            ncw = int(chunks_wc[w, c])
            if ncw == 0:
                continue
            base = cell_slot[(w, c)] // P
            wmeta[w]["gchunks"].extend(range(base, base + ncw))
            wmeta[w]["schunks"].append((c, ncw, base))

    # per-subcall true per-core index counts (num_idxs_reg): the SWDGE ucode
    # trims the trailing -1 pads, so padding and per-core count skew cost
    # neither descriptors nor DMA traffic.  The reg value must match the
    # trimmed count exactly (ring bookkeeping is sized from the register).
    # cells merged in PAIRS per subcall (adjacent in the chunk layout): the
    # first cell's pads are index-0 (mid-call, not trimmable), the second
    # cell's trailing -1 pads trim.  num_idxs_reg = ncw0*P + cnt(last cell).
    subcall_meta = []          # (c, [(w, cell_start, ncw), ...]) program order
    first_in_pair = set()      # (w, c) cells whose pads must be 0
    for gm in group_meta:
        for c, call in enumerate(gm["calls"]):
            if call["nchunks"] == 0:
                continue
            wc = [(w, cs, ncw) for w, (cs, ncw) in
                  zip(gm["windows"], call["cells"])]
            for i in range(0, len(wc), 2):
                pair = wc[i:i + 2]
                subcall_meta.append((c, pair))
                for (w, _, _) in pair[:-1]:
                    first_in_pair.add((w, c))
    NSUB = len(subcall_meta)
    gcnt = np.zeros((NCORES, NSUB), np.int32)
    for k, (c, pair) in enumerate(subcall_meta):
        full = sum(ncw for (_, _, ncw) in pair[:-1]) * P
        wl, _, ncwl = pair[-1]
        for core in range(NCORES):
            cidx = (core * WPC + wl) * NCLASS + c
            gcnt[core, k] = full + min(int(cellcnt[cidx]), ncwl * P)

    # per-core slot arrays; pads default -1 (trailing-trimmed), except cells
    # that are first in a merged pair, whose pads must be valid mid-call idx 0
    rel_all = rel2[order]
    idx_slots = np.full((NCORES, TOTCHUNKS * P), -1, np.int16)
    dst_slots = np.full((NCORES, TOTCHUNKS * P), 300.0, np.float32)
    for w in range(WPC):
        for c in range(NCLASS):
            ncw = int(chunks_wc[w, c])
            if ncw == 0:
                continue
            s0 = cell_slot[(w, c)]
            if (w, c) in first_in_pair:
                idx_slots[:, s0:s0 + ncw * P] = 0
            for core in range(NCORES):
                cidx = (core * WPC + w) * NCLASS + c
                cnt = int(cellcnt[cidx])
                st = int(cellstart[cidx])
                idx_slots[core, s0:s0 + cnt] = rel_all[st:st + cnt].astype(np.int16)
                dst_slots[core, s0:s0 + cnt] = (d2s[st:st + cnt] & (P - 1)).astype(np.float32)

    # wrapped int16 index tensors (per call: idx i at [i%16, i//16], tiled x8)
    idx16 = np.zeros((NCORES, 128, TOTCOLS), np.int16)
    for gm in group_meta:
        for call in gm["calls"]:
            cn = call["nchunks"]
            if cn == 0:
                continue
            s0 = call["chunk_start"] * P
            c0 = call["col_start"]
            seg = idx_slots[:, s0:s0 + cn * P]                  # [NCORES, n]
            wrapped = seg.reshape(NCORES, cn * P // 16, 16).transpose(0, 2, 1)
            idx16[:, :, c0:c0 + cn * P // 16] = np.tile(wrapped, (1, 8, 1))

    dstloc = dst_slots.reshape(NCORES, TOTCHUNKS, P).transpose(0, 2, 1)  # [NCORES,128,TOTCHUNKS]

    meta = dict(N=N, WPC=WPC, SHARD=SHARD, NPAD=NPAD,
                RS=RS.tolist(), RE=RE.tolist(), TSIZE=TSIZE,
                TOTCHUNKS=TOTCHUNKS, TOTCOLS=TOTCOLS, NSUB=NSUB,
                groups=group_meta, wmeta=wmeta,
                chunks_sig=chunks_wc.tobytes())
    return meta, deg, idx16, dstloc.astype(BF16), gcnt


# ------------------------------------------------------------- bass program


def _build_program(meta, IN_C, HID, OUT_C, debug_phase=None):
    import concourse.bacc as bacc
    import concourse.mybir as mybir
    import concourse.tile as tile

    WPC, SHARD, NPAD = meta["WPC"], meta["SHARD"], meta["NPAD"]
    RS, RE, TSIZE = meta["RS"], meta["RE"], meta["TSIZE"]
    TOTCHUNKS, TOTCOLS = meta["TOTCHUNKS"], meta["TOTCOLS"]
    KIN = IN_C // P

    nc = bacc.Bacc("TRN2", target_bir_lowering=False, debug=False,
                   num_devices=NCORES, num_swdge_queues=4)
    f32, bf16, i16, i32 = (mybir.dt.float32, mybir.dt.bfloat16,
                           mybir.dt.int16, mybir.dt.int32)

    zt_shard = nc.dram_tensor("zt_shard", [IN_C, SHARD], bf16, kind="ExternalInput").ap()
    gcnt = nc.dram_tensor("gcnt", [1, meta["NSUB"]], i32, kind="ExternalInput").ap()
    w1 = nc.dram_tensor("w1", [IN_C, HID], bf16, kind="ExternalInput").ap()
    w2 = nc.dram_tensor("w2", [HID, OUT_C], bf16, kind="ExternalInput").ap()
    idx16 = nc.dram_tensor("idx16", [128, TOTCOLS], i16, kind="ExternalInput").ap()
    dstloc = nc.dram_tensor("dstloc", [128, TOTCHUNKS], bf16, kind="ExternalInput").ap()
    dinv_col = nc.dram_tensor("dinv_col", [P, WPC], f32, kind="ExternalInput").ap()
    dinv2_col = nc.dram_tensor("dinv2_col", [P, WPC], f32, kind="ExternalInput").ap()
    sqd_row = nc.dram_tensor("sqd_row", [1, SHARD], bf16, kind="ExternalInput").ap()
    b1r = nc.dram_tensor("b1r", [1, HID], bf16, kind="ExternalInput").ap()
    b2r = nc.dram_tensor("b2r", [1, OUT_C], bf16, kind="ExternalInput").ap()
    out_shard = nc.dram_tensor("out_shard", [SHARD, OUT_C], f32, kind="ExternalOutput").ap()
    dbg = None
    if debug_phase is not None:
        dbg = nc.dram_tensor("dbg", [NPAD, HID], bf16, kind="ExternalOutput").ap()

    with tile.TileContext(nc) as tc:
        with (
            tc.tile_pool(name="dram", bufs=1, space="DRAM") as dram,
            tc.tile_pool(name="const", bufs=1) as cp,
        ):
            ag1_in = dram.tile([SHARD, HID], bf16)
            ag2_in = dram.tile([SHARD, HID], bf16)
            table1 = [dram.tile([TSIZE[r], HID], bf16, addr_space="Shared",
                                name=f"t1_{r}")
                      for r in range(NCLASS)]
            table2 = [dram.tile([TSIZE[r], HID], bf16, addr_space="Shared",
                                name=f"t2_{r}")
                      for r in range(NCLASS)]

            w1sb = cp.tile([P, KIN * HID], bf16)
            for ic in range(KIN):
                nc.sync.dma_start(w1sb[:, ic * HID:(ic + 1) * HID],
                                  w1[ic * P:(ic + 1) * P, :])
            w2sb = cp.tile([P, OUT_C], bf16)
            nc.sync.dma_start(w2sb[:], w2[:])
            dinvsb = cp.tile([P, WPC], f32)
            nc.sync.dma_start(dinvsb[:], dinv_col[:])
            dinv2sb = cp.tile([P, WPC], f32)
            nc.sync.dma_start(dinv2sb[:], dinv2_col[:])
            sqdsb = cp.tile([1, SHARD], bf16)
            nc.sync.dma_start(sqdsb[:], sqd_row[:])
            b1sb = cp.tile([1, HID], bf16)
            nc.sync.dma_start(b1sb[:], b1r[:])
            b2sb = cp.tile([1, OUT_C], bf16)
            nc.sync.dma_start(b2sb[:], b2r[:])
            gcntsb = cp.tile([1, meta["NSUB"]], i32)
            nc.sync.dma_start(gcntsb[:], gcnt[:])

            iota_i = cp.tile([P, P], i32)
            nc.gpsimd.iota(iota_i[:], pattern=[[1, P]], base=0, channel_multiplier=0)
            iota_bf = cp.tile([P, P], bf16)
            nc.vector.tensor_copy(iota_bf[:], iota_i[:])
            iotar_i = cp.tile([P, P], i32)
            nc.gpsimd.iota(iotar_i[:], pattern=[[0, P]], base=0, channel_multiplier=1)
            ident = cp.tile([P, P], bf16)
            nc.vector.tensor_tensor(out=ident[:], in0=iota_i[:], in1=iotar_i[:],
                                    op=mybir.AluOpType.is_equal)

            # ---------------- phase A: h1' = (z @ W1) * dinv  (own shard)
            with (
                tc.tile_pool(name="mmA", bufs=2) as mp,
                tc.tile_pool(name="psA", bufs=2, space="PSUM") as psA,
            ):
                ASTEP = 1024
                for t0 in range(0, SHARD, ASTEP):
                    gsz = min(ASTEP, SHARD - t0)
                    zts = []
                    for ic in range(KIN):
                        zt = mp.tile([P, gsz], bf16, tag=f"zt{ic}",
                                     padded_shape=[P, ASTEP], name=f"zt{ic}")
                        nc.sync.dma_start(
                            zt[:], zt_shard[ic * P:(ic + 1) * P, t0:t0 + gsz])
                        zts.append(zt)
                    for sub in range(gsz // P):
                        nt = t0 // P + sub
                        ps = psA.tile([P, HID], f32, name="psa")
                        for ic in range(KIN):
                            nc.tensor.matmul(
                                ps[:], lhsT=zts[ic][:, sub * P:(sub + 1) * P],
                                rhs=w1sb[:, ic * HID:(ic + 1) * HID],
                                start=(ic == 0), stop=(ic == KIN - 1))
                        hsb = mp.tile([P, HID], bf16, tag="hsb", name="hsb")
                        nc.scalar.mul(hsb[:], ps[:], dinvsb[:, nt:nt + 1])
                        nc.scalar.dma_start(ag1_in[nt * P:(nt + 1) * P, :], hsb[:])

            # chunked AllGathers: chunk r only needs window range [RS[r], RE[r])
            # of every core's phase-A output, so it fires while later ranges
            # are still being computed
            for r in range(NCLASS):
                nc.gpsimd.collective_compute(
                    "AllGather", mybir.AluOpType.bypass,
                    replica_groups=[list(range(NCORES))],
                    ins=[ag1_in[RS[r] * P:RE[r] * P, :]], outs=[table1[r][:]])

            # ---------------- aggregation layers
            cnt_regs = [nc.gpsimd.alloc_register(f"gcnt_q{q}") for q in range(NCLASS)]

            def agg_layer(table, layer, selfsrc, dbg_mode=None, post_group=None):
                subidx = itertools.count()
                with (
                    tc.tile_pool(name=f"gat{layer}", bufs=2) as gp,
                    tc.tile_pool(name=f"s{layer}", bufs=3) as sp,
                    tc.tile_pool(name=f"eps{layer}", bufs=3) as ep,
                    tc.tile_pool(name=f"ps{layer}", bufs=2, space="PSUM") as pp,
                    tc.tile_pool(name=f"pso{layer}", bufs=2, space="PSUM") as po,
                ):
                    maxgch = max(gm["nchunks"] for gm in meta["groups"])
                    maxgcol = max(gm["ncols"] for gm in meta["groups"])
                    maxsch = max(len(wm["gchunks"]) for wm in meta["wmeta"])
                    for gi, gm in enumerate(meta["groups"]):
                        gch, gcol = gm["nchunks"], gm["ncols"]
                        idx_sb = gp.tile([128, gcol], i16, tag="idx",
                                         padded_shape=[128, maxgcol], name="idx_sb")
                        nc.sync.dma_start(idx_sb[:], idx16[:, gm["col_base"]:gm["col_base"] + gcol])
                        dl_sb = gp.tile([P, gch], bf16, tag="dl",
                                        padded_shape=[P, maxgch], name="dl_sb")
                        nc.sync.dma_start(dl_sb[:], dstloc[:, gm["chunk_base"]:gm["chunk_base"] + gch])
                        # trimmed pad slots are never DMA-written: seed each of
                        # the two ring buffers ONCE over the FULL padded extent
                        # (later groups are larger!) so 0*garbage in the
                        # scatter matmul can't make NaN
                        galloc = maxgch if gi < 2 else gch
                        gbuf = gp.tile([P, galloc * P], bf16, tag="gbuf",
                                       padded_shape=[P, maxgch * P], name="gbuf")
                        if gi < 2:
                            nc.vector.memset(gbuf[:], 0)
                        # one subcall per (window, class) cell, ending at the
                        # cell boundary so trailing -1 pads get trimmed;
                        # round-robin across classes/queues so the sequencer
                        # never ring-blocks one queue before the other three
                        # Q7 pairs have work
                        subcalls = []
                        for c, call in enumerate(gm["calls"]):
                            if call["nchunks"] == 0:
                                continue
                            off0 = gm["chunk_base"]
                            cells = call["cells"]
                            cs = []
                            for i in range(0, len(cells), 2):
                                pair = cells[i:i + 2]
                                cell_start = pair[0][0]
                                sc = sum(ncw for (_, ncw) in pair)
                                k = next(subidx)
                                if sc == 0:
                                    continue
                                o = cell_start - off0
                                cs.append((c, o, (call["col_start"] - gm["col_base"])
                                           + (cell_start - call["chunk_start"]) * 8,
                                           sc, k))
                            subcalls.append(cs)
                        for tup in itertools.zip_longest(*subcalls):
                            for sub in tup:
                                if sub is None:
                                    continue
                                c, o, l0, sc, k = sub
                                nc.gpsimd.reg_load(cnt_regs[c], gcntsb[0:1, k:k + 1])
                                nc.gpsimd.dma_gather(
                                    out_ap=gbuf[:, o * P:(o + sc) * P]
                                        .rearrange("p (k f) -> p k f", f=P),
                                    in_ap=table[c][:],
                                    idxs_ap=idx_sb[:, l0:l0 + sc * 8],
                                    num_idxs=sc * P,
                                    num_idxs_reg=cnt_regs[c],
                                    elem_size=HID,
                                    single_packet=(sc <= 8),
                                    queue_num=c,
                                )
                        if dbg_mode == "gather":
                            # consume gbuf: copy first window-tile to ag2_in
                            gcp = ep.tile([P, HID], bf16, tag="l1", name="gcp")
                            nc.vector.tensor_copy(gcp[:], gbuf[:, :HID])
                            nc.sync.dma_start(
                                ag2_in[gm["windows"][0] * P:(gm["windows"][0] + 1) * P, :],
                                gcp[:])
                            continue
                        for w in gm["windows"]:
                            wm = meta["wmeta"][w]
                            cw = len(wm["gchunks"])
                            # own-window table rows for the self-loop term
                            tsw = ep.tile([P, HID], bf16, tag="tsw", name="tsw")
                            nc.scalar.dma_start(tsw[:], selfsrc[w * P:(w + 1) * P, :])
                            s_sb = sp.tile([P, max(cw, 1) * P], bf16, tag="s",
                                           padded_shape=[P, maxsch * P], name="s_sb")
                            soff = 0
                            for (c, ncw, gbase) in wm["schunks"]:
                                lc0 = gbase - gm["chunk_base"]
                                in0 = (dl_sb[:, lc0:lc0 + ncw]
                                       .rearrange("p (c one) -> p c one", one=1)
                                       .to_broadcast([P, ncw, P]))
                                in1 = (iota_bf[:]
                                       .rearrange("p (one j) -> p one j", one=1)
                                       .to_broadcast([P, ncw, P]))
                                nc.vector.tensor_tensor(
                                    out=s_sb[:, soff * P:(soff + ncw) * P]
                                        .rearrange("p (c j) -> p c j", j=P),
                                    in0=in0, in1=in1,
                                    op=mybir.AluOpType.is_equal)
                                soff += ncw
                            ps = pp.tile([P, P], f32, name="ps")
                            if layer == 1:
                                use_bias = dbg_mode != "nobias"
                                if use_bias:
                                    nc.tensor.matmul(
                                        ps[:], lhsT=sqdsb[:, w * P:(w + 1) * P],
                                        rhs=b1sb[:], start=True, stop=False)
                                nc.tensor.matmul(
                                    ps[:], lhsT=ident[:], rhs=tsw[:],
                                    start=(not use_bias), stop=(cw == 0))
                                for j, gc in enumerate(wm["gchunks"]):
                                    lgc = gc - gm["chunk_base"]
                                    nc.tensor.matmul(
                                        ps[:],
                                        lhsT=s_sb[:, j * P:(j + 1) * P],
                                        rhs=gbuf[:, lgc * P:(lgc + 1) * P],
                                        start=False,
                                        stop=(j == cw - 1))
                                l2row = ep.tile([P, HID], bf16, tag="l2r", name="l2row")
                                if dbg_mode == "nobias":
                                    nc.vector.tensor_copy(l2row[:], ps[:])
                                    nc.sync.dma_start(ag2_in[w * P:(w + 1) * P, :], l2row[:])
                                    continue
                                # dinv*relu(dinv*ps) == relu(dinv^2*ps): one op
                                nc.scalar.activation(
                                    l2row[:], ps[:],
                                    mybir.ActivationFunctionType.Relu,
                                    scale=dinv2sb[:, w:w + 1])
                                nc.scalar.dma_start(ag2_in[w * P:(w + 1) * P, :], l2row[:])
                            else:
                                # transposed accumulate: ps[f, d]
                                nc.tensor.matmul(
                                    ps[:], lhsT=tsw[:], rhs=ident[:],
                                    start=True, stop=(cw == 0))
                                for j, gc in enumerate(wm["gchunks"]):
                                    lgc = gc - gm["chunk_base"]
                                    nc.tensor.matmul(
                                        ps[:],
                                        lhsT=gbuf[:, lgc * P:(lgc + 1) * P],
                                        rhs=s_sb[:, j * P:(j + 1) * P],
                                        start=False, stop=(j == cw - 1))
                                a2t = ep.tile([P, P], bf16, tag="a2t", name="a2t")
                                nc.scalar.copy(a2t[:], ps[:])
                                ops = po.tile([P, OUT_C], f32, name="ops")
                                nc.tensor.matmul(ops[:], lhsT=a2t[:], rhs=w2sb[:],
                                                 start=True, stop=False)
                                nc.tensor.matmul(ops[:], lhsT=sqdsb[:, w * P:(w + 1) * P],
                                                 rhs=b2sb[:], start=False, stop=True)
                                fsb = ep.tile([P, OUT_C], f32, tag="fout", name="fsb")
                                nc.scalar.mul(fsb[:], ops[:], dinvsb[:, w:w + 1])
                                nc.sync.dma_start(out_shard[w * P:(w + 1) * P, :], fsb[:])
                        if post_group is not None:
                            post_group(gm["windows"][0])
                    if post_group is not None:
                        post_group(WPC)

            # issue layer-2's AllGather chunk r as soon as every window of
            # range r has been written (one-group hysteresis via first_window
            # of the group *currently* finishing)
            ag2_issued = [False] * NCLASS

            def issue_ag2(first_window_of_current):
                for r in range(NCLASS):
                    if not ag2_issued[r] and RE[r] <= first_window_of_current:
                        ag2_issued[r] = True
                        nc.gpsimd.collective_compute(
                            "AllGather", mybir.AluOpType.bypass,
                            replica_groups=[list(range(NCORES))],
                            ins=[ag2_in[RS[r] * P:RE[r] * P, :]],
                            outs=[table2[r][:]])

            if debug_phase == "A":
                for r in range(NCLASS):
                    nc.sync.dma_start(
                        dbg[sum(TSIZE[:r]):sum(TSIZE[:r + 1]), :], table1[r][:])
            else:
                agg_layer(table1, 1, ag1_in,
                          dbg_mode=debug_phase if debug_phase in ("gather", "nobias") else None,
                          post_group=None if debug_phase in ("C1", "gather", "nobias")
                          else issue_ag2)
                if debug_phase in ("C1", "gather", "nobias"):
                    nc.sync.dma_start(dbg[:SHARD, :], ag2_in[:])
                else:
                    agg_layer(table2, 2, ag2_in)

    nc.compile()
    return nc


# ----------------------------------------------------------------- entry


def _prepare_and_build(z, edge_index, W1, b1, W2, b2):
    N, IN_C = z.shape
    HID = W1.shape[1]
    OUT_C = W2.shape[1]
    meta, deg, idx16, dstloc, gcnt = _plan(edge_index, N)
    WPC, SHARD, NPAD = meta["WPC"], meta["SHARD"], meta["NPAD"]

    dinv = (1.0 / np.sqrt(deg)).astype(np.float32)
    dinv_pad = np.zeros(NPAD, np.float32)
    dinv_pad[:N] = dinv
    sqd_pad = np.zeros(NPAD, np.float32)
    sqd_pad[:N] = np.sqrt(deg).astype(np.float32)

    zpad = np.zeros((NPAD, IN_C), BF16)
    zpad[:N] = z.astype(BF16)

    w1b = np.ascontiguousarray(W1.astype(BF16))
    w2b = np.ascontiguousarray(W2.astype(BF16))
    b1b = np.ascontiguousarray(b1.reshape(1, HID).astype(BF16))
    b2b = np.ascontiguousarray(b2.reshape(1, OUT_C).astype(BF16))

    in_maps = []
    for c in range(NCORES):
        sl = slice(c * SHARD, (c + 1) * SHARD)
        in_maps.append({
            "zt_shard": np.ascontiguousarray(zpad[sl].T),
            "gcnt": np.ascontiguousarray(gcnt[c:c + 1]),
            "w1": w1b, "w2": w2b,
            "idx16": np.ascontiguousarray(idx16[c]),
            "dstloc": np.ascontiguousarray(dstloc[c]),
            "dinv_col": np.ascontiguousarray(dinv_pad[sl].reshape(WPC, P).T),
            "dinv2_col": np.ascontiguousarray((dinv_pad[sl] ** 2).reshape(WPC, P).T),
            "sqd_row": np.ascontiguousarray(sqd_pad[sl].reshape(1, SHARD).astype(BF16)),
            "b1r": b1b, "b2r": b2b,
        })

    cache_key = (N, IN_C, HID, OUT_C, meta["TOTCHUNKS"], hash(meta["chunks_sig"]))
    if cache_key in _PROGRAM_CACHE:
        nc = _PROGRAM_CACHE[cache_key]
    else:
        nc = _build_program(meta, IN_C, HID, OUT_C)
        _PROGRAM_CACHE[cache_key] = nc
    return nc, in_maps, meta


def _run(inputs, trace=False, trace_kwargs=None):
    from concourse.bass_utils import run_bass_kernel_spmd

    z = np.asarray(inputs["z"])
    edge_index = np.asarray(inputs["edge_index"])
    W1 = np.asarray(inputs["W1"])
    b1 = np.asarray(inputs["b1"])
    W2 = np.asarray(inputs["W2"])
    b2 = np.asarray(inputs["b2"])

    nc, in_maps, meta = _prepare_and_build(z, edge_index, W1, b1, W2, b2)
    res = run_bass_kernel_spmd(
        nc, in_maps, core_ids=list(range(NCORES)),
        trace=trace, **(trace_kwargs or {}))
    N = meta["N"]
    out = np.concatenate([r["out_shard"] for r in res.results], axis=0)[:N]
    return np.ascontiguousarray(out.astype(np.float32)), res


def kernel(**inputs):
    out, _ = _run(inputs, trace=False)
    return out



# revision 69
# speedup vs baseline: 1.2662x; 1.0196x over previous
"""Two-layer GCN (PyG GCNConv x2 with relu between) on 8 Trainium2 NeuronCores.

Math (per layer, A' = D^-1/2 (A + I) D^-1/2):
    h  = relu(A' (z @ W1) + b1)
    out = A' (h @ W2) + b2  ==  (A' h) @ W2 + b2      (aggregation commutes with the
                                                       feature-space linear map)
Both layers therefore aggregate 128-wide features only.

Distribution: nodes (and dst-partitioned edges) sharded across 8 cores;
weights replicated; per-layer AllGather of the (dinv-scaled) feature table in
bf16; per-core gather of source rows via bulk SWDGE dma_gather; segment-sum
realized as one-hot matmuls accumulating in PSUM.

The Bass program is specialized to the actual graph: per-(window, class)
chunk counts are compile-time constants derived from edge_index.
"""

import itertools

import numpy as np
import ml_dtypes

P = 128
NCORES = 8
NCLASS = 4          # src-range classes so relative gather indices fit int16
G = 4               # dst windows per gather group

BF16 = ml_dtypes.bfloat16

_PROGRAM_CACHE = {}


# ----------------------------------------------------------------- host prep


def _plan(edge_index, N):
    """Sort/partition edges; all compile-time metadata + per-core slot arrays.

    src "classes" are window-ranges: class r covers windows [RS[r], RE[r]) of
    every core.  The per-class table chunk (one AllGather each) is laid out
    [core, window-in-range, 128] so a chunk's AllGather can fire as soon as
    every core has produced that window range.
    """
    WPC = -(-N // (NCORES * P))            # windows per core
    SHARD = WPC * P
    NPAD = NCORES * SHARD

    # make the LAST range smallest: its AllGather chunk is the one whose
    # latency is exposed between phases
    last = max(WPC // NCLASS - 6, 1)
    rest = WPC - last
    b3, r3 = divmod(rest, NCLASS - 1)
    RW = np.array([b3 + (1 if r < r3 else 0) for r in range(NCLASS - 1)] + [last])
    RS = np.concatenate([[0], np.cumsum(RW)])[:NCLASS]
    RE = RS + RW
    TSIZE = (NCORES * RW * P).tolist()
    assert max(NCORES * RW * P) <= 32767
    range_of = np.repeat(np.arange(NCLASS), RW)          # [WPC] -> class

    src = np.asarray(edge_index[0], dtype=np.int64)
    dst = np.asarray(edge_index[1], dtype=np.int64)
    deg = np.bincount(dst, minlength=N).astype(np.float64) + 1.0

    # self-loops are added on-chip via an identity matmul over the local
    # window tile; only real edges go through the gather
    s2 = src
    d2 = dst

    def relidx(s):
        k = s // SHARD
        wloc = (s % SHARD) >> 7
        r = range_of[wloc]
        return (k * RW[r] + (wloc - RS[r])) * P + (s & (P - 1)), r

    win = d2 >> 7
    rel2, cls = relidx(s2)
    key = win * NCLASS + cls
    # secondary sort by table row: each SDMA engine then walks ascending
    # addresses within a gather call (HBM locality)
    order = np.lexsort((rel2, key))
    d2s = d2[order]

    NW = NPAD // P
    cellcnt = np.bincount(key, minlength=NW * NCLASS)
    cellstart = np.concatenate([[0], np.cumsum(cellcnt)]).astype(np.int64)
    counts_core = cellcnt.reshape(NCORES, WPC, NCLASS)
    chunks_wc = -(-counts_core.max(axis=0) // P)      # [WPC, NCLASS]

    groups = [list(range(g, min(g + G, WPC))) for g in range(0, WPC, G)]

    # global chunk layout: for each group, for each class, for each window in
    # group, that window's class chunks (one contiguous dma_gather per
    # (group, class)).
    group_meta = []           # per group: dict with chunk/col offsets
    wmeta = [dict(schunks=[], gchunks=[]) for _ in range(WPC)]
    chunkpos = 0
    colpos = 0
    cell_slot = {}            # (w, c) -> global slot start
    for grp in groups:
        g_chunk_base = chunkpos
        g_col_base = colpos
        calls = []
        for c in range(NCLASS):
            call_chunk_start = chunkpos
            call_col_start = colpos
            cells = []
            for w in grp:
                ncw = int(chunks_wc[w, c])
                cell_slot[(w, c)] = chunkpos * P
                cells.append((chunkpos, ncw))
                chunkpos += ncw
            cn = chunkpos - call_chunk_start
            colpos += cn * P // 16
            calls.append(dict(chunk_start=call_chunk_start, nchunks=cn,
                              col_start=call_col_start, ncols=colpos - call_col_start,
                              cells=cells))
        group_meta.append(dict(chunk_base=g_chunk_base, nchunks=chunkpos - g_chunk_base,
                               col_base=g_col_base, ncols=colpos - g_col_base,
                               calls=calls, windows=list(grp)))
    TOTCHUNKS = chunkpos
    TOTCOLS = colpos

    # per-window ordered chunk lists: s-order (class-major) + matching global
    # chunk ids, and per-(w,c) count for S generation
    for w in range(WPC):
        for c in range(NCLASS):
            ncw = int(chunks_wc[w, c])
            if ncw == 0:
                continue
            base = cell_slot[(w, c)] // P
            wmeta[w]["gchunks"].extend(range(base, base + ncw))
            wmeta[w]["schunks"].append((c, ncw, base))

    # per-subcall true per-core index counts (num_idxs_reg): the SWDGE ucode
    # trims the trailing -1 pads, so padding and per-core count skew cost
    # neither descriptors nor DMA traffic.  The reg value must match the
    # trimmed count exactly (ring bookkeeping is sized from the register).
    # cells merged in PAIRS per subcall (adjacent in the chunk layout): the
    # first cell's pads are index-0 (mid-call, not trimmable), the second
    # cell's trailing -1 pads trim.  num_idxs_reg = ncw0*P + cnt(last cell).
    subcall_meta = []          # (c, [(w, cell_start, ncw), ...]) program order
    first_in_pair = set()      # (w, c) cells whose pads must be 0
    for gm in group_meta:
        for c, call in enumerate(gm["calls"]):
            if call["nchunks"] == 0:
                continue
            wc = [(w, cs, ncw) for w, (cs, ncw) in
                  zip(gm["windows"], call["cells"])]
            for i in range(0, len(wc), 2):
                pair = wc[i:i + 2]
                subcall_meta.append((c, pair))
                for (w, _, _) in pair[:-1]:
                    first_in_pair.add((w, c))
    NSUB = len(subcall_meta)
    gcnt = np.zeros((NCORES, NSUB), np.int32)
    for k, (c, pair) in enumerate(subcall_meta):
        full = sum(ncw for (_, _, ncw) in pair[:-1]) * P
        wl, _, ncwl = pair[-1]
        for core in range(NCORES):
            cidx = (core * WPC + wl) * NCLASS + c
            gcnt[core, k] = full + min(int(cellcnt[cidx]), ncwl * P)

    # per-core slot arrays; pads default -1 (trailing-trimmed), except cells
    # that are first in a merged pair, whose pads must be valid mid-call idx 0
    rel_all = rel2[order]
    idx_slots = np.full((NCORES, TOTCHUNKS * P), -1, np.int16)
    dst_slots = np.full((NCORES, TOTCHUNKS * P), 300.0, np.float32)
    for w in range(WPC):
        for c in range(NCLASS):
            ncw = int(chunks_wc[w, c])
            if ncw == 0:
                continue
            s0 = cell_slot[(w, c)]
            if (w, c) in first_in_pair:
                idx_slots[:, s0:s0 + ncw * P] = 0
            for core in range(NCORES):
                cidx = (core * WPC + w) * NCLASS + c
                cnt = int(cellcnt[cidx])
                st = int(cellstart[cidx])
                idx_slots[core, s0:s0 + cnt] = rel_all[st:st + cnt].astype(np.int16)
                dst_slots[core, s0:s0 + cnt] = (d2s[st:st + cnt] & (P - 1)).astype(np.float32)

    # wrapped int16 index tensors (per call: idx i at [i%16, i//16], tiled x8)
    idx16 = np.zeros((NCORES, 128, TOTCOLS), np.int16)
    for gm in group_meta:
        for call in gm["calls"]:
            cn = call["nchunks"]
            if cn == 0:
                continue
            s0 = call["chunk_start"] * P
            c0 = call["col_start"]
            seg = idx_slots[:, s0:s0 + cn * P]                  # [NCORES, n]
            wrapped = seg.reshape(NCORES, cn * P // 16, 16).transpose(0, 2, 1)
            idx16[:, :, c0:c0 + cn * P // 16] = np.tile(wrapped, (1, 8, 1))

    dstloc = dst_slots.reshape(NCORES, TOTCHUNKS, P).transpose(0, 2, 1)  # [NCORES,128,TOTCHUNKS]

    meta = dict(N=N, WPC=WPC, SHARD=SHARD, NPAD=NPAD,
                RS=RS.tolist(), RE=RE.tolist(), TSIZE=TSIZE,
                TOTCHUNKS=TOTCHUNKS, TOTCOLS=TOTCOLS, NSUB=NSUB,
                groups=group_meta, wmeta=wmeta,
                chunks_sig=chunks_wc.tobytes())
    return meta, deg, idx16, dstloc.astype(BF16), gcnt


# ------------------------------------------------------------- bass program


def _build_program(meta, IN_C, HID, OUT_C, debug_phase=None):
    import concourse.bacc as bacc
    import concourse.mybir as mybir
    import concourse.tile as tile

    WPC, SHARD, NPAD = meta["WPC"], meta["SHARD"], meta["NPAD"]
    RS, RE, TSIZE = meta["RS"], meta["RE"], meta["TSIZE"]
    TOTCHUNKS, TOTCOLS = meta["TOTCHUNKS"], meta["TOTCOLS"]
    KIN = IN_C // P

    nc = bacc.Bacc("TRN2", target_bir_lowering=False, debug=False,
                   num_devices=NCORES, num_swdge_queues=4)
    f32, bf16, i16, i32 = (mybir.dt.float32, mybir.dt.bfloat16,
                           mybir.dt.int16, mybir.dt.int32)

    zt_shard = nc.dram_tensor("zt_shard", [IN_C, SHARD], bf16, kind="ExternalInput").ap()
    gcnt = nc.dram_tensor("gcnt", [1, meta["NSUB"]], i32, kind="ExternalInput").ap()
    w1 = nc.dram_tensor("w1", [IN_C, HID], bf16, kind="ExternalInput").ap()
    w2 = nc.dram_tensor("w2", [HID, OUT_C], bf16, kind="ExternalInput").ap()
    idx16 = nc.dram_tensor("idx16", [128, TOTCOLS], i16, kind="ExternalInput").ap()
    dstloc = nc.dram_tensor("dstloc", [128, TOTCHUNKS], bf16, kind="ExternalInput").ap()
    dinv_col = nc.dram_tensor("dinv_col", [P, WPC], f32, kind="ExternalInput").ap()
    dinv2_col = nc.dram_tensor("dinv2_col", [P, WPC], f32, kind="ExternalInput").ap()
    sqd_row = nc.dram_tensor("sqd_row", [1, SHARD], bf16, kind="ExternalInput").ap()
    b1r = nc.dram_tensor("b1r", [1, HID], bf16, kind="ExternalInput").ap()
    b2r = nc.dram_tensor("b2r", [1, OUT_C], bf16, kind="ExternalInput").ap()
    out_shard = nc.dram_tensor("out_shard", [SHARD, OUT_C], f32, kind="ExternalOutput").ap()
    dbg = None
    if debug_phase is not None:
        dbg = nc.dram_tensor("dbg", [NPAD, HID], bf16, kind="ExternalOutput").ap()

    with tile.TileContext(nc) as tc:
        with (
            tc.tile_pool(name="dram", bufs=1, space="DRAM") as dram,
            tc.tile_pool(name="const", bufs=1) as cp,
        ):
            ag1_in = dram.tile([SHARD, HID], bf16)
            ag2_in = dram.tile([SHARD, HID], bf16)
            table1 = [dram.tile([TSIZE[r], HID], bf16, addr_space="Shared",
                                name=f"t1_{r}")
                      for r in range(NCLASS)]
            table2 = [dram.tile([TSIZE[r], HID], bf16, addr_space="Shared",
                                name=f"t2_{r}")
                      for r in range(NCLASS)]

            w1sb = cp.tile([P, KIN * HID], bf16)
            for ic in range(KIN):
                nc.sync.dma_start(w1sb[:, ic * HID:(ic + 1) * HID],
                                  w1[ic * P:(ic + 1) * P, :])
            w2sb = cp.tile([P, OUT_C], bf16)
            nc.sync.dma_start(w2sb[:], w2[:])
            dinvsb = cp.tile([P, WPC], f32)
            nc.sync.dma_start(dinvsb[:], dinv_col[:])
            dinv2sb = cp.tile([P, WPC], f32)
            nc.sync.dma_start(dinv2sb[:], dinv2_col[:])
            sqdsb = cp.tile([1, SHARD], bf16)
            nc.sync.dma_start(sqdsb[:], sqd_row[:])
            b1sb = cp.tile([1, HID], bf16)
            nc.sync.dma_start(b1sb[:], b1r[:])
            b2sb = cp.tile([1, OUT_C], bf16)
            nc.sync.dma_start(b2sb[:], b2r[:])
            gcntsb = cp.tile([1, meta["NSUB"]], i32)
            nc.sync.dma_start(gcntsb[:], gcnt[:])

            iota_i = cp.tile([P, P], i32)
            nc.gpsimd.iota(iota_i[:], pattern=[[1, P]], base=0, channel_multiplier=0)
            iota_bf = cp.tile([P, P], bf16)
            nc.vector.tensor_copy(iota_bf[:], iota_i[:])
            iotar_i = cp.tile([P, P], i32)
            nc.gpsimd.iota(iotar_i[:], pattern=[[0, P]], base=0, channel_multiplier=1)
            ident = cp.tile([P, P], bf16)
            nc.vector.tensor_tensor(out=ident[:], in0=iota_i[:], in1=iotar_i[:],
                                    op=mybir.AluOpType.is_equal)

            # ---------------- phase A: h1' = (z @ W1) * dinv  (own shard)
            with (
                tc.tile_pool(name="mmA", bufs=3) as mp,
                tc.tile_pool(name="psA", bufs=4, space="PSUM") as psA,
            ):
                ASTEP = 1024
                for t0 in range(0, SHARD, ASTEP):
                    gsz = min(ASTEP, SHARD - t0)
                    zts = []
                    for ic in range(KIN):
                        zt = mp.tile([P, gsz], bf16, tag=f"zt{ic}",
                                     padded_shape=[P, ASTEP], name=f"zt{ic}")
                        nc.sync.dma_start(
                            zt[:], zt_shard[ic * P:(ic + 1) * P, t0:t0 + gsz])
                        zts.append(zt)
                    for sub in range(gsz // P):
                        nt = t0 // P + sub
                        ps = psA.tile([P, HID], f32, name="psa")
                        for ic in range(KIN):
                            nc.tensor.matmul(
                                ps[:], lhsT=zts[ic][:, sub * P:(sub + 1) * P],
                                rhs=w1sb[:, ic * HID:(ic + 1) * HID],
                                start=(ic == 0), stop=(ic == KIN - 1))
                        hsb = mp.tile([P, HID], bf16, tag="hsb", name="hsb")
                        nc.scalar.mul(hsb[:], ps[:], dinvsb[:, nt:nt + 1])
                        nc.scalar.dma_start(ag1_in[nt * P:(nt + 1) * P, :], hsb[:])

            # chunked AllGathers: chunk r only needs window range [RS[r], RE[r])
            # of every core's phase-A output, so it fires while later ranges
            # are still being computed
            for r in range(NCLASS):
                nc.gpsimd.collective_compute(
                    "AllGather", mybir.AluOpType.bypass,
                    replica_groups=[list(range(NCORES))],
                    ins=[ag1_in[RS[r] * P:RE[r] * P, :]], outs=[table1[r][:]])

            # ---------------- aggregation layers
            cnt_regs = [nc.gpsimd.alloc_register(f"gcnt_q{q}") for q in range(NCLASS)]

            def agg_layer(table, layer, selfsrc, dbg_mode=None, post_group=None):
                subidx = itertools.count()
                qrr = itertools.count()
                with (
                    tc.tile_pool(name=f"gat{layer}", bufs=2) as gp,
                    tc.tile_pool(name=f"s{layer}", bufs=3) as sp,
                    tc.tile_pool(name=f"eps{layer}", bufs=3) as ep,
                    tc.tile_pool(name=f"ps{layer}", bufs=2, space="PSUM") as pp,
                    tc.tile_pool(name=f"pso{layer}", bufs=2, space="PSUM") as po,
                ):
                    maxgch = max(gm["nchunks"] for gm in meta["groups"])
                    maxgcol = max(gm["ncols"] for gm in meta["groups"])
                    maxsch = max(len(wm["gchunks"]) for wm in meta["wmeta"])
                    for gi, gm in enumerate(meta["groups"]):
                        gch, gcol = gm["nchunks"], gm["ncols"]
                        idx_sb = gp.tile([128, gcol], i16, tag="idx",
                                         padded_shape=[128, maxgcol], name="idx_sb")
                        nc.sync.dma_start(idx_sb[:], idx16[:, gm["col_base"]:gm["col_base"] + gcol])
                        dl_sb = gp.tile([P, gch], bf16, tag="dl",
                                        padded_shape=[P, maxgch], name="dl_sb")
                        nc.sync.dma_start(dl_sb[:], dstloc[:, gm["chunk_base"]:gm["chunk_base"] + gch])
                        # trimmed pad slots are never DMA-written: seed each of
                        # the two ring buffers ONCE over the FULL padded extent
                        # (later groups are larger!) so 0*garbage in the
                        # scatter matmul can't make NaN
                        galloc = maxgch if gi < 2 else gch
                        gbuf = gp.tile([P, galloc * P], bf16, tag="gbuf",
                                       padded_shape=[P, maxgch * P], name="gbuf")
                        if gi < 2:
                            nc.vector.memset(gbuf[:], 0)
                        # one subcall per (window, class) cell, ending at the
                        # cell boundary so trailing -1 pads get trimmed;
                        # round-robin across classes/queues so the sequencer
                        # never ring-blocks one queue before the other three
                        # Q7 pairs have work
                        subcalls = []
                        for c, call in enumerate(gm["calls"]):
                            if call["nchunks"] == 0:
                                continue
                            off0 = gm["chunk_base"]
                            cells = call["cells"]
                            cs = []
                            for i in range(0, len(cells), 2):
                                pair = cells[i:i + 2]
                                cell_start = pair[0][0]
                                sc = sum(ncw for (_, ncw) in pair)
                                k = next(subidx)
                                if sc == 0:
                                    continue
                                o = cell_start - off0
                                cs.append((c, o, (call["col_start"] - gm["col_base"])
                                           + (cell_start - call["chunk_start"]) * 8,
                                           sc, k))
                            subcalls.append(cs)
                        for tup in itertools.zip_longest(*subcalls):
                            for sub in tup:
                                if sub is None:
                                    continue
                                c, o, l0, sc, k = sub
                                # queue decoupled from class: perfect Q7-pair
                                # load balance despite uneven range widths
                                q = next(qrr) % NCLASS
                                nc.gpsimd.reg_load(cnt_regs[q], gcntsb[0:1, k:k + 1])
                                nc.gpsimd.dma_gather(
                                    out_ap=gbuf[:, o * P:(o + sc) * P]
                                        .rearrange("p (k f) -> p k f", f=P),
                                    in_ap=table[c][:],
                                    idxs_ap=idx_sb[:, l0:l0 + sc * 8],
                                    num_idxs=sc * P,
                                    num_idxs_reg=cnt_regs[q],
                                    elem_size=HID,
                                    single_packet=(sc <= 8),
                                    queue_num=q,
                                )
                        if dbg_mode == "gather":
                            # consume gbuf: copy first window-tile to ag2_in
                            gcp = ep.tile([P, HID], bf16, tag="l1", name="gcp")
                            nc.vector.tensor_copy(gcp[:], gbuf[:, :HID])
                            nc.sync.dma_start(
                                ag2_in[gm["windows"][0] * P:(gm["windows"][0] + 1) * P, :],
                                gcp[:])
                            continue
                        for w in gm["windows"]:
                            wm = meta["wmeta"][w]
                            cw = len(wm["gchunks"])
                            # own-window table rows for the self-loop term
                            tsw = ep.tile([P, HID], bf16, tag="tsw", name="tsw")
                            nc.scalar.dma_start(tsw[:], selfsrc[w * P:(w + 1) * P, :])
                            s_sb = sp.tile([P, max(cw, 1) * P], bf16, tag="s",
                                           padded_shape=[P, maxsch * P], name="s_sb")
                            soff = 0
                            for (c, ncw, gbase) in wm["schunks"]:
                                lc0 = gbase - gm["chunk_base"]
                                in0 = (dl_sb[:, lc0:lc0 + ncw]
                                       .rearrange("p (c one) -> p c one", one=1)
                                       .to_broadcast([P, ncw, P]))
                                in1 = (iota_bf[:]
                                       .rearrange("p (one j) -> p one j", one=1)
                                       .to_broadcast([P, ncw, P]))
                                nc.vector.tensor_tensor(
                                    out=s_sb[:, soff * P:(soff + ncw) * P]
                                        .rearrange("p (c j) -> p c j", j=P),
                                    in0=in0, in1=in1,
                                    op=mybir.AluOpType.is_equal)
                                soff += ncw
                            ps = pp.tile([P, P], f32, name="ps")
                            if layer == 1:
                                use_bias = dbg_mode != "nobias"
                                if use_bias:
                                    nc.tensor.matmul(
                                        ps[:], lhsT=sqdsb[:, w * P:(w + 1) * P],
                                        rhs=b1sb[:], start=True, stop=False)
                                nc.tensor.matmul(
                                    ps[:], lhsT=ident[:], rhs=tsw[:],
                                    start=(not use_bias), stop=(cw == 0))
                                for j, gc in enumerate(wm["gchunks"]):
                                    lgc = gc - gm["chunk_base"]
                                    nc.tensor.matmul(
                                        ps[:],
                                        lhsT=s_sb[:, j * P:(j + 1) * P],
                                        rhs=gbuf[:, lgc * P:(lgc + 1) * P],
                                        start=False,
                                        stop=(j == cw - 1))
                                l2row = ep.tile([P, HID], bf16, tag="l2r", name="l2row")
                                if dbg_mode == "nobias":
                                    nc.vector.tensor_copy(l2row[:], ps[:])
                                    nc.sync.dma_start(ag2_in[w * P:(w + 1) * P, :], l2row[:])
                                    continue
                                # dinv*relu(dinv*ps) == relu(dinv^2*ps): one op
                                nc.scalar.activation(
                                    l2row[:], ps[:],
                                    mybir.ActivationFunctionType.Relu,
                                    scale=dinv2sb[:, w:w + 1])
                                nc.scalar.dma_start(ag2_in[w * P:(w + 1) * P, :], l2row[:])
                            else:
                                # transposed accumulate: ps[f, d]
                                nc.tensor.matmul(
                                    ps[:], lhsT=tsw[:], rhs=ident[:],
                                    start=True, stop=(cw == 0))
                                for j, gc in enumerate(wm["gchunks"]):
                                    lgc = gc - gm["chunk_base"]
                                    nc.tensor.matmul(
                                        ps[:],
                                        lhsT=gbuf[:, lgc * P:(lgc + 1) * P],
                                        rhs=s_sb[:, j * P:(j + 1) * P],
                                        start=False, stop=(j == cw - 1))
                                a2t = ep.tile([P, P], bf16, tag="a2t", name="a2t")
                                nc.scalar.copy(a2t[:], ps[:])
                                ops = po.tile([P, OUT_C], f32, name="ops")
                                nc.tensor.matmul(ops[:], lhsT=a2t[:], rhs=w2sb[:],
                                                 start=True, stop=False)
                                nc.tensor.matmul(ops[:], lhsT=sqdsb[:, w * P:(w + 1) * P],
                                                 rhs=b2sb[:], start=False, stop=True)
                                fsb = ep.tile([P, OUT_C], f32, tag="fout", name="fsb")
                                nc.scalar.mul(fsb[:], ops[:], dinvsb[:, w:w + 1])
                                nc.sync.dma_start(out_shard[w * P:(w + 1) * P, :], fsb[:])
                        if post_group is not None:
                            post_group(gm["windows"][0])
                    if post_group is not None:
                        post_group(WPC)

            # issue layer-2's AllGather chunk r as soon as every window of
            # range r has been written (one-group hysteresis via first_window
            # of the group *currently* finishing)
            ag2_issued = [False] * NCLASS

            def issue_ag2(first_window_of_current):
                for r in range(NCLASS):
                    if not ag2_issued[r] and RE[r] <= first_window_of_current:
                        ag2_issued[r] = True
                        nc.gpsimd.collective_compute(
                            "AllGather", mybir.AluOpType.bypass,
                            replica_groups=[list(range(NCORES))],
                            ins=[ag2_in[RS[r] * P:RE[r] * P, :]],
                            outs=[table2[r][:]])

            if debug_phase == "A":
                for r in range(NCLASS):
                    nc.sync.dma_start(
                        dbg[sum(TSIZE[:r]):sum(TSIZE[:r + 1]), :], table1[r][:])
            else:
                agg_layer(table1, 1, ag1_in,
                          dbg_mode=debug_phase if debug_phase in ("gather", "nobias") else None,
                          post_group=None if debug_phase in ("C1", "gather", "nobias")
                          else issue_ag2)
                if debug_phase in ("C1", "gather", "nobias"):
                    nc.sync.dma_start(dbg[:SHARD, :], ag2_in[:])
                else:
                    agg_layer(table2, 2, ag2_in)

    nc.compile()
    return nc


# ----------------------------------------------------------------- entry


def _prepare_and_build(z, edge_index, W1, b1, W2, b2):
    N, IN_C = z.shape
    HID = W1.shape[1]
    OUT_C = W2.shape[1]
    meta, deg, idx16, dstloc, gcnt = _plan(edge_index, N)
    WPC, SHARD, NPAD = meta["WPC"], meta["SHARD"], meta["NPAD"]

    dinv = (1.0 / np.sqrt(deg)).astype(np.float32)
    dinv_pad = np.zeros(NPAD, np.float32)
    dinv_pad[:N] = dinv
    sqd_pad = np.zeros(NPAD, np.float32)
    sqd_pad[:N] = np.sqrt(deg).astype(np.float32)

    zpad = np.zeros((NPAD, IN_C), BF16)
    zpad[:N] = z.astype(BF16)

    w1b = np.ascontiguousarray(W1.astype(BF16))
    w2b = np.ascontiguousarray(W2.astype(BF16))
    b1b = np.ascontiguousarray(b1.reshape(1, HID).astype(BF16))
    b2b = np.ascontiguousarray(b2.reshape(1, OUT_C).astype(BF16))

    in_maps = []
    for c in range(NCORES):
        sl = slice(c * SHARD, (c + 1) * SHARD)
        in_maps.append({
            "zt_shard": np.ascontiguousarray(zpad[sl].T),
            "gcnt": np.ascontiguousarray(gcnt[c:c + 1]),
            "w1": w1b, "w2": w2b,
            "idx16": np.ascontiguousarray(idx16[c]),
            "dstloc": np.ascontiguousarray(dstloc[c]),
            "dinv_col": np.ascontiguousarray(dinv_pad[sl].reshape(WPC, P).T),
            "dinv2_col": np.ascontiguousarray((dinv_pad[sl] ** 2).reshape(WPC, P).T),
            "sqd_row": np.ascontiguousarray(sqd_pad[sl].reshape(1, SHARD).astype(BF16)),
            "b1r": b1b, "b2r": b2b,
        })

    cache_key = (N, IN_C, HID, OUT_C, meta["TOTCHUNKS"], hash(meta["chunks_sig"]))
    if cache_key in _PROGRAM_CACHE:
        nc = _PROGRAM_CACHE[cache_key]
    else:
        nc = _build_program(meta, IN_C, HID, OUT_C)
        _PROGRAM_CACHE[cache_key] = nc
    return nc, in_maps, meta


def _run(inputs, trace=False, trace_kwargs=None):
    from concourse.bass_utils import run_bass_kernel_spmd

    z = np.asarray(inputs["z"])
    edge_index = np.asarray(inputs["edge_index"])
    W1 = np.asarray(inputs["W1"])
    b1 = np.asarray(inputs["b1"])
    W2 = np.asarray(inputs["W2"])
    b2 = np.asarray(inputs["b2"])

    nc, in_maps, meta = _prepare_and_build(z, edge_index, W1, b1, W2, b2)
    res = run_bass_kernel_spmd(
        nc, in_maps, core_ids=list(range(NCORES)),
        trace=trace, **(trace_kwargs or {}))
    N = meta["N"]
    out = np.concatenate([r["out_shard"] for r in res.results], axis=0)[:N]
    return np.ascontiguousarray(out.astype(np.float32)), res


def kernel(**inputs):
    out, _ = _run(inputs, trace=False)
    return out

